# revision 21
# baseline (speedup 1.0000x reference)
"""MoE feed-forward (8 experts, hard argmin routing) on 8 TRN2 NeuronCores.

Strategy
--------
Host (numpy): rms_norm + argmin routing, then a dispatch plan at 32-token
granularity: tokens sorted by expert, packed into a UNIFORM per-core
structure of K expert-segments (same sizes on every core; only the data
-- which expert's weights, which tokens -- differs per core).  An exact
cover search (DP over experts) minimizes the per-core token-slot count T.

Numerics: bf16 weights/activations (fp32 PSUM accumulate) for most of the
contraction, with a configurable slice of the contraction computed in
fp8(e4m3) using DoubleRow matmuls (2 k-tiles per instruction -> 2x PE
throughput on that slice, measured 221ns for K=256 N=512 vs 222ns bf16
K=128).  fp8 operands are pre-scaled by powers of 2 (exact) to dodge
e4m3's tiny subnormal range; the down-proj's act scale is folded into the
up-proj "a" weight columns so the device applies it for free.

Device (Bass/Tile, SPMD x8): per segment, weights stream through SBUF in
512-column chunks, each feeding matmul rounds right after it lands.
up-proj -> swiglu (ACT Silu + DVE mul, fp8 slice written as e4m3) ->
down-proj, yT written back to DRAM in fp32 per 2-dout-tile round.

Host: scatter y back to token order and add the skip connection.
"""

import json

import ml_dtypes
import numpy as np

N_EXPERTS = 8
DIM = 1024
HID = 2048
N_CORES = 8
P = 128
EPS = 1e-6
G = 32          # token granularity of the dispatch plan

# fp8 config: number of DoubleRow pairs on each matmul's contraction.
N8U = 1         # up:   N8U pairs of ko-tiles (of 4 pairs = 1024 contraction)
N8D = 1         # down: N8D pairs of kh-tiles (of 8 pairs = 2048 contraction)
SU = 16.0       # up fp8 weight pre-scale (xn fp8 slice divided by SU)
SD = 8.0        # down fp8 weight pre-scale (act fp8 slice divided by SD)
KOB = 8 - 2 * N8U    # bf16 ko-tiles (up)
KHB = 16 - 2 * N8D   # bf16 kh-tiles (down)
WARMUP = 48

BF16 = ml_dtypes.bfloat16
FP8 = ml_dtypes.float8_e4m3fn


def _q8(v):
    return np.clip(v, -240.0, 240.0).astype(FP8)


# ----------------------------------------------------------------------------
# BIR fixup: walrus in this container accepts at most ONE sync-wait per
# instruction.  Split instructions with k>1 waits into (k-1) pure-wait
# EventSemaphore instructions on the same engine immediately before.
# ----------------------------------------------------------------------------
def _split_multiwait_json(bir_bytes: bytes) -> bytes:
    m = json.loads(bir_bytes)
    ctr = 0
    for func in m["functions"]:
        for bb in func["blocks"]:
            out = []
            for inst in bb["instructions"]:
                si = inst.get("sync_info")
                waits = (si or {}).get("on_wait") or []
                if len(waits) > 1:
                    for w in waits[:-1]:
                        ctr += 1
                        out.append({
                            "debug": inst.get("debug", 0),
                            "engine": inst["engine"],
                            "ins": [],
                            "outs": [],
                            "name": f"waitfix_{ctr}",
                            "opcode": "EventSemaphore",
                            "sync_info": {"on_update": [], "on_wait": [w]},
                        })
                    si["on_wait"] = [waits[-1]]
                out.append(inst)
            bb["instructions"] = out
    return json.dumps(m).encode()


def _patch_bass_json(nc):
    orig = nc.to_json_bytes

    def patched():
        return _split_multiwait_json(orig())

    nc.to_json_bytes = patched


# ----------------------------------------------------------------------------
# Host-side routing (replicates the reference numerics in fp32)
# ----------------------------------------------------------------------------
def _route(x, scale, centroids):
    xf = x.reshape(-1, DIM).astype(np.float32)
    ms = np.mean(xf * xf, axis=-1, keepdims=True)
    s = scale.astype(np.float32) / np.sqrt(ms + EPS)
    xn = xf * s
    nx = np.sum(xn * xn, axis=-1)[:, None]
    ny = np.sum(centroids * centroids, axis=-1)[None, :]
    d2 = nx + ny - 2.0 * (xn @ centroids.T)
    ids = np.argmin(d2, axis=-1).astype(np.int32)
    return xn, ids


# ----------------------------------------------------------------------------
# Dispatch planner: uniform comp across cores, exact cover, 32-token units
# ----------------------------------------------------------------------------
def _compositions(total, k):
    if k == 1:
        yield (total,)
        return
    for first in range((total + k - 1) // k, total - k + 2):
        for rest in _compositions(total - first, k - 1):
            if rest[0] <= first:
                yield (first,) + rest


def _cover(comp, units):
    """comp: slot sizes (units), 8 slots each. Returns {e: counts per pos}."""
    K = len(comp)
    experts = sorted(range(len(units)), key=lambda e: -units[e])
    avail = [N_CORES] * K

    def rec(i):
        if i == len(experts):
            return {}
        e = experts[i]
        need = units[e]
        if need == 0:
            rest = rec(i + 1)
            if rest is not None:
                rest[e] = (0,) * K
            return rest
        opts = []
        max_counts = [min(avail[j], (need + comp[j] - 1) // comp[j])
                      for j in range(K)]

        def enum(j, counts, cap):
            if cap >= need:
                if all(c == 0 or cap - comp[k2] < need
                       for k2, c in enumerate(counts)):
                    opts.append((cap - need, tuple(counts)))
                return
            if j == K:
                return
            for c in range(max_counts[j] + 1):
                counts[j] = c
                enum(j + 1, counts, cap + c * comp[j])
                if cap + c * comp[j] >= need:
                    break
            counts[j] = 0

        enum(0, [0] * K, 0)
        opts.sort()
        for _, counts in opts[:60]:
            for j in range(K):
                avail[j] -= counts[j]
            rest = rec(i + 1)
            for j in range(K):
                avail[j] += counts[j]
            if rest is not None:
                rest[e] = counts
                return rest
        return None

    return rec(0)


def _seg_cost(gn):
    """PE-time model (ns) for one token group of gn tokens."""
    n_up = KOB * 2 + N8U * 2
    ldw_up = KOB * 2 * 107 + N8U * 2 * 214
    n_dn = KHB * 2 + N8D * 2
    ldw_dn = KHB * 2 * 107 + N8D * 2 * 214
    up = 16 * max(n_up * (gn / 2.4 + 2.5), ldw_up)
    dn = 4 * max(n_dn * (gn / 2.4 + 2.5), ldw_dn)
    return up + dn


def _comp_cost(comp_units):
    c = 0.0
    for u in comp_units:
        stok = u * G
        while stok > 0:
            gn = min(512, stok)
            stok -= gn
            c += _seg_cost(gn)
    return c


def _plan(ids):
    cnt = np.bincount(ids, minlength=N_EXPERTS)
    units = [int((c + G - 1) // G) for c in cnt]
    total = sum(units)
    lo = (total + N_CORES - 1) // N_CORES
    found = None
    for T in range(lo, lo + 40):
        cands = []
        for K in (2, 3, 4):
            cands += list(_compositions(T, K))
        # try cheapest comps first (cost model includes a per-segment
        # penalty for the extra weight DMA); first feasible wins
        cands.sort(key=lambda comp: _comp_cost(comp) + 2000.0 * len(comp))
        for comp in cands:
            sol = _cover(comp, units)
            if sol is not None:
                found = (comp, sol)
                break
        if found:
            break
    if not found:
        raise RuntimeError("dispatch packing failed")
    comp, sol = found
    # largest segment first: its down phase starts latest, giving the
    # (bandwidth-limited) startup DMA time to deliver the down weights;
    # smallest last so the final output-DMA tail is short.
    order = np.argsort([-c for c in comp], kind="stable")
    comp = tuple(comp[j] for j in order)
    sol = {e: tuple(s[j] for j in order) for e, s in sol.items()}
    # slots: position j -> list of experts (len 8, None = unused)
    tok_by_e = [np.where(ids == e)[0] for e in range(N_EXPERTS)]
    slot_expert = {}
    for j in range(len(comp)):
        lst = []
        for e in range(N_EXPERTS):
            lst += [e] * sol[e][j]
        assert len(lst) <= N_CORES
        lst += [None] * (N_CORES - len(lst))
        for c in range(N_CORES):
            slot_expert[(c, j)] = lst[c]
    # fill tokens: per expert, slots ordered by capacity desc
    comp_tok = tuple(c * G for c in comp)
    cursor = [0] * N_EXPERTS
    chunks = {}
    for e in range(N_EXPERTS):
        slots = [(c, j) for (c, j), ee in slot_expert.items() if ee == e]
        slots.sort(key=lambda s: (-comp_tok[s[1]], s[0]))
        for (c, j) in slots:
            take = min(comp_tok[j], len(tok_by_e[e]) - cursor[e])
            take = max(take, 0)
            chunks[(c, j)] = take
            cursor[e] += take
    for e in range(N_EXPERTS):
        assert cursor[e] == len(tok_by_e[e]), "plan did not cover all tokens"
    return comp_tok, slot_expert, chunks, tok_by_e


# ----------------------------------------------------------------------------
# Device program
# ----------------------------------------------------------------------------
def _build_program(comp_tok):
    import concourse.bass as bass
    import concourse.mybir as mybir
    import concourse.tile as tile

    f32 = mybir.dt.float32
    bf16 = mybir.dt.bfloat16
    fp8 = mybir.dt.float8e4
    Silu = mybir.ActivationFunctionType.Silu
    DR = mybir.MatmulPerfMode.DoubleRow

    K = len(comp_tok)
    T = sum(comp_tok)
    NCH = 8  # 512-col chunks of the 4096 up output dim

    nc = bass.Bass("TRN2", debug=False)
    xnb_in = nc.dram_tensor("xnb", [P, KOB, T], bf16, kind="ExternalInput").ap()
    xn8_in = nc.dram_tensor("xn8", [P, 2 * N8U, T], fp8,
                            kind="ExternalInput").ap()
    upc_in = nc.dram_tensor("upc", [K, NCH, P, KOB, 512], bf16,
                            kind="ExternalInput").ap()
    up8_in = nc.dram_tensor("up8", [K, P, 2 * N8U, 4096], fp8,
                            kind="ExternalInput").ap()
    dnb_in = nc.dram_tensor("dnb", [K, P, KHB, 1024], bf16,
                            kind="ExternalInput").ap()
    dn8_in = nc.dram_tensor("dn8", [K, P, 2 * N8D, 1024], fp8,
                            kind="ExternalInput").ap()
    yt_out = nc.dram_tensor("yt", [P, 8, T], f32, kind="ExternalOutput").ap()

    with tile.TileContext(nc) as tc:
        with (
            tc.tile_pool(name="upc", bufs=12) as upc_pool,
            tc.tile_pool(name="up8", bufs=2) as up8_pool,
            tc.tile_pool(name="dnb", bufs=1) as dnb_pool,
            tc.tile_pool(name="dn8", bufs=2) as dn8_pool,
            tc.tile_pool(name="xnb", bufs=1) as xnb_pool,
            tc.tile_pool(name="xn8", bufs=1) as xn8_pool,
            tc.tile_pool(name="act", bufs=2) as act_pool,
            tc.tile_pool(name="act8", bufs=2) as act8_pool,
            tc.tile_pool(name="yc", bufs=4) as yc_pool,
            tc.tile_pool(name="warm", bufs=1) as warm_pool,
            tc.tile_pool(name="ps", bufs=8, space="PSUM") as ps,
        ):
            # PE warm-up: dependency-free matmuls on a zeroed scratch tile
            # keep PE busy while the first DMAs land.
            wsrc = warm_pool.tile([P, 256], bf16, tag="warm")
            nc.gpsimd.memset(wsrc[:], 0.0)
            wps = [ps.tile([P, P], f32, tag="ps", name=f"wps{i}")
                   for i in range(2)]
            for i in range(WARMUP):
                nc.tensor.matmul(wps[i % 2][:], wsrc[:, 0:P],
                                 wsrc[:, P:2 * P], start=True, stop=True)

            xnb_t = xnb_pool.tile([P, KOB, T], bf16, tag="xnb")
            xn8_t = xn8_pool.tile([P, 2 * N8U, T], fp8, tag="xn8")

            up8_tiles = {}
            upc_tiles = {}

            def push_seg_up(s, fine=False):
                w8 = up8_pool.tile([P, 2 * N8U, 4096], fp8, tag="up8",
                                   name=f"up8_{s}")
                up8_tiles[s] = w8
                for c in range(NCH):
                    wt = upc_pool.tile([P, KOB, 512], bf16, tag="upc",
                                       name=f"upc_{s}_{c}")
                    upc_tiles[(s, c)] = wt
                if fine:
                    # startup-critical ordering: round-0 weights first, the
                    # fp8 tile in quarters right behind the chunks they feed
                    def q8push(i):
                        nc.sync.dma_start(
                            up8_tiles[s][:, :, 1024 * i:1024 * (i + 1)],
                            up8_in[s, :, :, 1024 * i:1024 * (i + 1)])

                    nc.gpsimd.dma_start(upc_tiles[(s, 0)][:, :, 0:256],
                                        upc_in[s, 0, :, :, 0:256])
                    nc.sync.dma_start(upc_tiles[(s, 0)][:, :, 256:512],
                                      upc_in[s, 0, :, :, 256:512])
                    q8push(0)
                    gate_push()
                    nc.sync.dma_start(upc_tiles[(s, 1)][:], upc_in[s, 1])
                    q8push(1)
                    nc.sync.dma_start(upc_tiles[(s, 2)][:], upc_in[s, 2])
                    nc.sync.dma_start(upc_tiles[(s, 3)][:], upc_in[s, 3])
                    q8push(2)
                    nc.sync.dma_start(upc_tiles[(s, 4)][:], upc_in[s, 4])
                    nc.sync.dma_start(upc_tiles[(s, 5)][:], upc_in[s, 5])
                    q8push(3)
                    nc.sync.dma_start(upc_tiles[(s, 6)][:], upc_in[s, 6])
                    nc.sync.dma_start(upc_tiles[(s, 7)][:], upc_in[s, 7])
                else:
                    nc.sync.dma_start(w8[:], up8_in[s])
                    for c in range(NCH):
                        nc.sync.dma_start(upc_tiles[(s, c)][:], upc_in[s, c])

            # initial DMA pushes (program order = sync-engine issue order;
            # all transfers share one ~400GB/s queue, so order = priority):
            # seg0 first tokens -> seg0 up weights -> seg0 down -> the rest.
            # first pushes go out on idle engines in parallel so the sync
            # engine's ~0.65us-per-descriptor serialization doesn't delay
            # the startup-critical transfers
            c0 = min(512, comp_tok[0])
            nc.scalar.dma_start(xnb_t[:, :, 0:c0], xnb_in[:, :, 0:c0])
            nc.gpsimd.dma_start(xn8_t[:, :, 0:c0], xn8_in[:, :, 0:c0])
            # the DMA queues fair-share HBM bandwidth, so the sync engine
            # must not push the non-critical transfers until the first
            # round's tiles have landed: an SBUF->SBUF dma that READS the
            # xnb tile stalls the sync queue on exactly that condition
            dly = warm_pool.tile([P, 64], bf16, tag="dly")

            def gate_push():
                nc.sync.dma_start(dly[:], xnb_t[:, 0, 0:64])

            push_seg_up(0, fine=True)
            dnb0 = dnb_pool.tile([P, KHB, 1024], bf16, tag="dnb", name="dnb_0")
            nc.sync.dma_start(dnb0[:], dnb_in[0])
            dnb_tiles = {0: dnb0}
            d8 = dn8_pool.tile([P, 2 * N8D, 1024], fp8, tag="dn8", name="dn8_0")
            nc.sync.dma_start(d8[:], dn8_in[0])
            dn8_tiles = {0: d8}
            if c0 < T:
                nc.sync.dma_start(xnb_t[:, :, c0:T], xnb_in[:, :, c0:T])
                nc.sync.dma_start(xn8_t[:, :, c0:T], xn8_in[:, :, c0:T])

            col = 0
            for s in range(K):
                # down bf16 weights: bufs=1, pushed here (s>0) so the WAR
                # wait on the previous segment's last down matmul is met.
                if s > 0:
                    dnbt = dnb_pool.tile([P, KHB, 1024], bf16, tag="dnb",
                                         name=f"dnb_{s}")
                    nc.sync.dma_start(dnbt[:], dnb_in[s])
                    dnb_tiles[s] = dnbt
                dnbt = dnb_tiles[s]

                stok = comp_tok[s]
                rem = stok
                segoff = 0
                while rem > 0:
                    gn = min(512, rem)
                    rem -= gn
                    is_last_group = rem == 0
                    xslc = slice(col + segoff, col + segoff + gn)
                    segoff += gn
                    act_t = act_pool.tile([P, 16, gn], bf16, tag="act")
                    act8_t = act8_pool.tile([P, 2 * N8D, gn], fp8, tag="act8")
                    # ---- up projection: 16 rounds of (a, g) pairs ----
                    for j in range(16):
                        ch = upc_tiles[(s, j // 2)]
                        off = (j % 2) * 256
                        pa = ps.tile([P, gn], f32, tag="ps", name="pa")
                        pg = ps.tile([P, gn], f32, tag="ps", name="pg")
                        for ko in range(KOB):
                            first = ko == 0
                            last = N8U == 0 and ko == KOB - 1
                            nc.tensor.matmul(pa[:], ch[:, ko, off:off + P],
                                             xnb_t[:, ko, xslc],
                                             start=first, stop=last)
                            nc.tensor.matmul(pg[:],
                                             ch[:, ko, off + P:off + 256],
                                             xnb_t[:, ko, xslc],
                                             start=first, stop=last)
                        w8 = up8_tiles[s]
                        for pr in range(N8U):
                            first = KOB == 0 and pr == 0
                            last = pr == N8U - 1
                            kk = slice(2 * pr, 2 * pr + 2)
                            ca = j * 256
                            nc.tensor.matmul(pa[:], w8[:, kk, ca:ca + P],
                                             xn8_t[:, kk, xslc],
                                             start=first, stop=last,
                                             perf_mode=DR)
                            nc.tensor.matmul(pg[:],
                                             w8[:, kk, ca + P:ca + 256],
                                             xn8_t[:, kk, xslc],
                                             start=first, stop=last,
                                             perf_mode=DR)
                        # swiglu: act = pa * silu(pg); fp8 kh's go to act8
                        nc.scalar.activation(act_t[:, j, :], pg[:], Silu)
                        if j < KHB:
                            nc.vector.tensor_mul(act_t[:, j, :], pa[:],
                                                 act_t[:, j, :])
                        else:
                            nc.vector.tensor_mul(act8_t[:, j - KHB, :], pa[:],
                                                 act_t[:, j, :])
                        # prefetch next segment's up weights as their chunk
                        # buffers free up (after this group's last use)
                        if is_last_group and s + 1 < K and j == 15:
                            push_seg_up(s + 1)
                            d8n = dn8_pool.tile([P, 2 * N8D, 1024], fp8,
                                                tag="dn8", name=f"dn8_{s+1}")
                            nc.sync.dma_start(d8n[:], dn8_in[s + 1])
                            dn8_tiles[s + 1] = d8n
                    # ---- down projection: 4 rounds x 2 dout tiles ----
                    d8t = dn8_tiles[s]
                    for rr in range(4):
                        pd = [ps.tile([P, gn], f32, tag="ps", name=f"pd{q}")
                              for q in range(2)]
                        for kh in range(KHB):
                            first = kh == 0
                            last = N8D == 0 and kh == KHB - 1
                            for q in range(2):
                                cc = (2 * rr + q) * P
                                nc.tensor.matmul(pd[q][:],
                                                 dnbt[:, kh, cc:cc + P],
                                                 act_t[:, kh, :],
                                                 start=first, stop=last)
                        for pr in range(N8D):
                            first = KHB == 0 and pr == 0
                            last = pr == N8D - 1
                            kk = slice(2 * pr, 2 * pr + 2)
                            for q in range(2):
                                cc = (2 * rr + q) * P
                                nc.tensor.matmul(pd[q][:],
                                                 d8t[:, kk, cc:cc + P],
                                                 act8_t[:, kk, :],
                                                 start=first, stop=last,
                                                 perf_mode=DR)
                        yc = yc_pool.tile([P, 2, gn], f32, tag="yc")
                        if s == K - 1 and is_last_group and rr == 3:
                            # final round: split the copy across DVE + ACT
                            # and DMA per half so the drain tail is short
                            nc.vector.tensor_copy(yc[:, 0, :], pd[0][:])
                            nc.sync.dma_start(yt_out[:, 6, xslc], yc[:, 0, :])
                            nc.scalar.activation(
                                yc[:, 1, :], pd[1][:],
                                mybir.ActivationFunctionType.Copy)
                            nc.sync.dma_start(yt_out[:, 7, xslc], yc[:, 1, :])
                        else:
                            for q in range(2):
                                nc.vector.tensor_copy(yc[:, q, :], pd[q][:])
                            nc.sync.dma_start(
                                yt_out[:, 2 * rr:2 * rr + 2, xslc], yc[:])
                col += stok

    _patch_bass_json(nc)
    return nc


# ----------------------------------------------------------------------------
# Host-side weight packing
# ----------------------------------------------------------------------------
def _pack_up(up_e):
    """up_w[e] [DIM, 2H] f32 -> (upc [8, P, KOB, 512] bf16,
    up8 [P, 2*N8U, 4096] fp8)."""
    Wd = np.empty((DIM, 4096), dtype=np.float32)
    for j in range(16):
        a = up_e[:, j * P:(j + 1) * P]
        if j >= KHB:
            a = a * (1.0 / SD)
        Wd[:, j * 256:j * 256 + P] = a
        Wd[:, j * 256 + P:(j + 1) * 256] = up_e[:, HID + j * P:HID + (j + 1) * P]
    ub = Wd[:KOB * P].astype(BF16).reshape(KOB, P, NCHU, 512)
    upc = np.ascontiguousarray(ub.transpose(2, 1, 0, 3))
    u8 = _q8(Wd[KOB * P:] * SU).reshape(2 * N8U, P, 4096)
    up8 = np.ascontiguousarray(u8.transpose(1, 0, 2))
    return upc, up8


NCHU = 8


def _pack_dn(dn_e):
    """down_w[e] [HID, DIM] f32 -> (dnb [P, KHB, 1024] bf16,
    dn8 [P, 2*N8D, 1024] fp8)."""
    db = dn_e[:KHB * P].astype(BF16).reshape(KHB, P, DIM)
    dnb = np.ascontiguousarray(db.transpose(1, 0, 2))
    d8 = _q8(dn_e[KHB * P:] * SD).reshape(2 * N8D, P, DIM)
    dn8 = np.ascontiguousarray(d8.transpose(1, 0, 2))
    return dnb, dn8


# ----------------------------------------------------------------------------
# Entry point
# ----------------------------------------------------------------------------
def _run(inputs, trace=False, tmpdir=None):
    from concourse.bass_utils import run_bass_kernel_spmd

    x = np.asarray(inputs["x"])
    scale = np.asarray(inputs["scale"])
    centroids = np.asarray(inputs["centroids"])
    up_w = np.asarray(inputs["up_w"])
    down_w = np.asarray(inputs["down_w"])

    B, S, D = x.shape
    ntok = B * S
    xf32 = x.reshape(ntok, D).astype(np.float32)

    xn, ids = _route(x, scale, centroids)
    comp_tok, slot_expert, chunks, tok_by_e = _plan(ids)
    K = len(comp_tok)
    T = sum(comp_tok)

    up_packed = {}
    dn_packed = {}
    for e in range(N_EXPERTS):
        if any(ee == e for ee in slot_expert.values()):
            up_packed[e] = _pack_up(up_w[e].astype(np.float32))
            dn_packed[e] = _pack_dn(down_w[e].astype(np.float32))

    xnT = np.ascontiguousarray(xn.T)  # [DIM, ntok] f32
    cursor = [0] * N_EXPERTS
    core_cols_tok = [np.zeros(T, dtype=np.int64) for _ in range(N_CORES)]
    core_cols_valid = [np.zeros(T, dtype=bool) for _ in range(N_CORES)]
    in_maps = []
    # fill order must match _plan's chunk assignment (capacity desc, core asc)
    fill_order = {}
    for e in range(N_EXPERTS):
        slots = [(c, j) for (c, j), ee in slot_expert.items() if ee == e]
        slots.sort(key=lambda s: (-comp_tok[s[1]], s[0]))
        fill_order[e] = slots
    seg_start = np.concatenate([[0], np.cumsum(comp_tok)])
    for e in range(N_EXPERTS):
        for (c, j) in fill_order[e]:
            take = chunks[(c, j)]
            if take:
                sel = tok_by_e[e][cursor[e]:cursor[e] + take]
                cursor[e] += take
                a = int(seg_start[j])
                core_cols_tok[c][a:a + take] = sel
                core_cols_valid[c][a:a + take] = True

    for c in range(N_CORES):
        upc = np.zeros((K, NCHU, P, KOB, 512), dtype=BF16)
        up8 = np.zeros((K, P, 2 * N8U, 4096), dtype=FP8)
        dnb = np.zeros((K, P, KHB, 1024), dtype=BF16)
        dn8 = np.zeros((K, P, 2 * N8D, 1024), dtype=FP8)
        for j in range(K):
            e = slot_expert[(c, j)]
            if e is not None:
                upc[j], up8[j] = up_packed[e]
                dnb[j], dn8[j] = dn_packed[e]
        xcols = xnT[:, core_cols_tok[c]]  # [DIM, T] f32 (invalid cols garbage)
        xcols = xcols * core_cols_valid[c][None, :]
        xnb = np.ascontiguousarray(
            xcols[:KOB * P].astype(BF16).reshape(KOB, P, T).transpose(1, 0, 2))
        xn8 = np.ascontiguousarray(
            _q8(xcols[KOB * P:] * (1.0 / SU)).reshape(2 * N8U, P, T)
            .transpose(1, 0, 2))
        in_maps.append({"xnb": xnb, "xn8": xn8, "upc": upc, "up8": up8,
                        "dnb": dnb, "dn8": dn8})

    nc = _build_program(comp_tok)
    kwargs = {}
    if trace:
        kwargs = dict(trace=True, tmpdir=tmpdir)
    res = run_bass_kernel_spmd(nc, in_maps, core_ids=list(range(N_CORES)),
                               **kwargs)

    # ---- scatter + skip ----
    out = xf32.copy()
    for c in range(N_CORES):
        yt = np.ascontiguousarray(
            res.results[c]["yt"].reshape(P, 8, T).transpose(1, 0, 2)
        ).reshape(8 * P, T)  # [DIM, T]
        valid = core_cols_valid[c]
        toks = core_cols_tok[c][valid]
        out[toks] = xf32[toks] + yt[:, valid].T
    return out.reshape(B, S, D).astype(x.dtype), res


def kernel(**inputs) -> np.ndarray:
    out, _ = _run(inputs)
    return out


# revision 23
# speedup vs baseline: 1.2265x; 1.2265x over previous
"""MoE feed-forward (8 experts, hard argmin routing) on 8 TRN2 NeuronCores.

Strategy
--------
Host (numpy): rms_norm + argmin routing, then a dispatch plan at 32-token
granularity: tokens sorted by expert, packed into a UNIFORM per-core
structure of K expert-segments (same sizes on every core; only the data
-- which expert's weights, which tokens -- differs per core).  An exact
cover search (DP over experts) minimizes the per-core token-slot count T.

Numerics: bf16 weights/activations (fp32 PSUM accumulate) for most of the
contraction, with a configurable slice of the contraction computed in
fp8(e4m3) using DoubleRow matmuls (2 k-tiles per instruction -> 2x PE
throughput on that slice, measured 221ns for K=256 N=512 vs 222ns bf16
K=128).  fp8 operands are pre-scaled by powers of 2 (exact) to dodge
e4m3's tiny subnormal range; the down-proj's act scale is folded into the
up-proj "a" weight columns so the device applies it for free.

Device (Bass/Tile, SPMD x8): per segment, weights stream through SBUF in
512-column chunks, each feeding matmul rounds right after it lands.
up-proj -> swiglu (ACT Silu + DVE mul, fp8 slice written as e4m3) ->
down-proj, yT written back to DRAM in fp32 per 2-dout-tile round.

Host: scatter y back to token order and add the skip connection.
"""

import json

import ml_dtypes
import numpy as np

N_EXPERTS = 8
DIM = 1024
HID = 2048
N_CORES = 8
P = 128
EPS = 1e-6
G = 32          # token granularity of the dispatch plan

# fp8 config: number of DoubleRow pairs on each matmul's contraction.
N8U = 1         # up:   N8U pairs of ko-tiles (of 4 pairs = 1024 contraction)
N8D = 1         # down: N8D pairs of kh-tiles (of 8 pairs = 2048 contraction)
SU = 16.0       # up fp8 weight pre-scale (xn fp8 slice divided by SU)
SD = 8.0        # down fp8 weight pre-scale (act fp8 slice divided by SD)
KOB = 8 - 2 * N8U    # bf16 ko-tiles (up)
KHB = 16 - 2 * N8D   # bf16 kh-tiles (down)
WARMUP = 48

BF16 = ml_dtypes.bfloat16
FP8 = ml_dtypes.float8_e4m3fn


def _q8(v):
    return np.clip(v, -240.0, 240.0).astype(FP8)


# ----------------------------------------------------------------------------
# BIR fixup: walrus in this container accepts at most ONE sync-wait per
# instruction.  Split instructions with k>1 waits into (k-1) pure-wait
# EventSemaphore instructions on the same engine immediately before.
# ----------------------------------------------------------------------------
def _split_multiwait_json(bir_bytes: bytes) -> bytes:
    m = json.loads(bir_bytes)
    ctr = 0
    for func in m["functions"]:
        for bb in func["blocks"]:
            out = []
            for inst in bb["instructions"]:
                si = inst.get("sync_info")
                waits = (si or {}).get("on_wait") or []
                if len(waits) > 1:
                    for w in waits[:-1]:
                        ctr += 1
                        out.append({
                            "debug": inst.get("debug", 0),
                            "engine": inst["engine"],
                            "ins": [],
                            "outs": [],
                            "name": f"waitfix_{ctr}",
                            "opcode": "EventSemaphore",
                            "sync_info": {"on_update": [], "on_wait": [w]},
                        })
                    si["on_wait"] = [waits[-1]]
                out.append(inst)
            bb["instructions"] = out
    return json.dumps(m).encode()


def _patch_bass_json(nc):
    orig = nc.to_json_bytes

    def patched():
        return _split_multiwait_json(orig())

    nc.to_json_bytes = patched


# ----------------------------------------------------------------------------
# Host-side routing (replicates the reference numerics in fp32)
# ----------------------------------------------------------------------------
def _route(x, scale, centroids):
    xf = x.reshape(-1, DIM).astype(np.float32)
    ms = np.mean(xf * xf, axis=-1, keepdims=True)
    s = scale.astype(np.float32) / np.sqrt(ms + EPS)
    xn = xf * s
    nx = np.sum(xn * xn, axis=-1)[:, None]
    ny = np.sum(centroids * centroids, axis=-1)[None, :]
    d2 = nx + ny - 2.0 * (xn @ centroids.T)
    ids = np.argmin(d2, axis=-1).astype(np.int32)
    return xn, ids


# ----------------------------------------------------------------------------
# Dispatch planner: uniform comp across cores, exact cover, 32-token units
# ----------------------------------------------------------------------------
def _compositions(total, k):
    if k == 1:
        yield (total,)
        return
    for first in range((total + k - 1) // k, total - k + 2):
        for rest in _compositions(total - first, k - 1):
            if rest[0] <= first:
                yield (first,) + rest


def _cover(comp, units):
    """comp: slot sizes (units), 8 slots each. Returns {e: counts per pos}."""
    K = len(comp)
    experts = sorted(range(len(units)), key=lambda e: -units[e])
    avail = [N_CORES] * K

    def rec(i):
        if i == len(experts):
            return {}
        e = experts[i]
        need = units[e]
        if need == 0:
            rest = rec(i + 1)
            if rest is not None:
                rest[e] = (0,) * K
            return rest
        opts = []
        max_counts = [min(avail[j], (need + comp[j] - 1) // comp[j])
                      for j in range(K)]

        def enum(j, counts, cap):
            if cap >= need:
                if all(c == 0 or cap - comp[k2] < need
                       for k2, c in enumerate(counts)):
                    opts.append((cap - need, tuple(counts)))
                return
            if j == K:
                return
            for c in range(max_counts[j] + 1):
                counts[j] = c
                enum(j + 1, counts, cap + c * comp[j])
                if cap + c * comp[j] >= need:
                    break
            counts[j] = 0

        enum(0, [0] * K, 0)
        opts.sort()
        for _, counts in opts[:60]:
            for j in range(K):
                avail[j] -= counts[j]
            rest = rec(i + 1)
            for j in range(K):
                avail[j] += counts[j]
            if rest is not None:
                rest[e] = counts
                return rest
        return None

    return rec(0)


def _seg_cost(gn):
    """PE-time model (ns) for one token group of gn tokens."""
    n_up = KOB * 2 + N8U * 2
    ldw_up = KOB * 2 * 107 + N8U * 2 * 214
    n_dn = KHB * 2 + N8D * 2
    ldw_dn = KHB * 2 * 107 + N8D * 2 * 214
    up = 16 * max(n_up * (gn / 2.4 + 2.5), ldw_up)
    dn = 4 * max(n_dn * (gn / 2.4 + 2.5), ldw_dn)
    return up + dn


def _comp_cost(comp_units):
    c = 0.0
    for u in comp_units:
        stok = u * G
        while stok > 0:
            gn = min(512, stok)
            stok -= gn
            c += _seg_cost(gn)
    return c


def _plan(ids):
    cnt = np.bincount(ids, minlength=N_EXPERTS)
    units = [int((c + G - 1) // G) for c in cnt]
    total = sum(units)
    lo = (total + N_CORES - 1) // N_CORES
    found = None
    for T in range(lo, lo + 40):
        cands = []
        for K in (2, 3, 4):
            cands += list(_compositions(T, K))
        # try cheapest comps first (cost model includes a per-segment
        # penalty for the extra weight DMA); first feasible wins
        cands.sort(key=lambda comp: _comp_cost(comp) + 2000.0 * len(comp))
        for comp in cands:
            sol = _cover(comp, units)
            if sol is not None:
                found = (comp, sol)
                break
        if found:
            break
    if not found:
        raise RuntimeError("dispatch packing failed")
    comp, sol = found
    # largest segment first: its down phase starts latest, giving the
    # (bandwidth-limited) startup DMA time to deliver the down weights;
    # smallest last so the final output-DMA tail is short.
    order = np.argsort([-c for c in comp], kind="stable")
    comp = tuple(comp[j] for j in order)
    sol = {e: tuple(s[j] for j in order) for e, s in sol.items()}
    # slots: position j -> list of experts (len 8, None = unused)
    tok_by_e = [np.where(ids == e)[0] for e in range(N_EXPERTS)]
    slot_expert = {}
    for j in range(len(comp)):
        lst = []
        for e in range(N_EXPERTS):
            lst += [e] * sol[e][j]
        assert len(lst) <= N_CORES
        lst += [None] * (N_CORES - len(lst))
        for c in range(N_CORES):
            slot_expert[(c, j)] = lst[c]
    # fill tokens: per expert, slots ordered by capacity desc
    comp_tok = tuple(c * G for c in comp)
    cursor = [0] * N_EXPERTS
    chunks = {}
    for e in range(N_EXPERTS):
        slots = [(c, j) for (c, j), ee in slot_expert.items() if ee == e]
        slots.sort(key=lambda s: (-comp_tok[s[1]], s[0]))
        for (c, j) in slots:
            take = min(comp_tok[j], len(tok_by_e[e]) - cursor[e])
            take = max(take, 0)
            chunks[(c, j)] = take
            cursor[e] += take
    for e in range(N_EXPERTS):
        assert cursor[e] == len(tok_by_e[e]), "plan did not cover all tokens"
    return comp_tok, slot_expert, chunks, tok_by_e


# ----------------------------------------------------------------------------
# Device program
# ----------------------------------------------------------------------------
def _build_program(comp_tok):
    import concourse.bass as bass
    import concourse.mybir as mybir
    import concourse.tile as tile

    f32 = mybir.dt.float32
    bf16 = mybir.dt.bfloat16
    fp8 = mybir.dt.float8e4
    Silu = mybir.ActivationFunctionType.Silu
    DR = mybir.MatmulPerfMode.DoubleRow

    K = len(comp_tok)
    T = sum(comp_tok)
    NCH = 8  # 512-col chunks of the 4096 up output dim

    nc = bass.Bass("TRN2", debug=False)
    xnb_in = nc.dram_tensor("xnb", [P, KOB, T], bf16, kind="ExternalInput").ap()
    xn8_in = nc.dram_tensor("xn8", [P, 2 * N8U, T], fp8,
                            kind="ExternalInput").ap()
    upc_in = nc.dram_tensor("upc", [K, NCH, P, KOB, 512], bf16,
                            kind="ExternalInput").ap()
    up8_in = nc.dram_tensor("up8", [K, P, 2 * N8U, 4096], fp8,
                            kind="ExternalInput").ap()
    dnb_in = nc.dram_tensor("dnb", [K, P, KHB, 1024], bf16,
                            kind="ExternalInput").ap()
    dn8_in = nc.dram_tensor("dn8", [K, P, 2 * N8D, 1024], fp8,
                            kind="ExternalInput").ap()
    yt_out = nc.dram_tensor("yt", [P, 8, T], f32, kind="ExternalOutput").ap()

    with tile.TileContext(nc) as tc:
        with (
            tc.tile_pool(name="upc", bufs=12) as upc_pool,
            tc.tile_pool(name="up8", bufs=2) as up8_pool,
            tc.tile_pool(name="dnb", bufs=1) as dnb_pool,
            tc.tile_pool(name="dn8", bufs=2) as dn8_pool,
            tc.tile_pool(name="xnb", bufs=1) as xnb_pool,
            tc.tile_pool(name="xn8", bufs=1) as xn8_pool,
            tc.tile_pool(name="act", bufs=2) as act_pool,
            tc.tile_pool(name="act8", bufs=2) as act8_pool,
            tc.tile_pool(name="yc", bufs=4) as yc_pool,
            tc.tile_pool(name="warm", bufs=1) as warm_pool,
            tc.tile_pool(name="ps", bufs=8, space="PSUM") as ps,
        ):
            # PE warm-up: dependency-free matmuls on a zeroed scratch tile
            # keep PE busy while the first DMAs land.
            wsrc = warm_pool.tile([P, 256], bf16, tag="warm")
            nc.gpsimd.memset(wsrc[:], 0.0)
            wps = [ps.tile([P, P], f32, tag="ps", name=f"wps{i}")
                   for i in range(2)]
            for i in range(WARMUP):
                nc.tensor.matmul(wps[i % 2][:], wsrc[:, 0:P],
                                 wsrc[:, P:2 * P], start=True, stop=True)

            xnb_t = xnb_pool.tile([P, KOB, T], bf16, tag="xnb")
            xn8_t = xn8_pool.tile([P, 2 * N8U, T], fp8, tag="xn8")

            up8_tiles = {}
            upc_tiles = {}

            def push_seg_up(s, fine=False):
                w8 = up8_pool.tile([P, 2 * N8U, 4096], fp8, tag="up8",
                                   name=f"up8_{s}")
                up8_tiles[s] = w8
                for c in range(NCH):
                    wt = upc_pool.tile([P, KOB, 512], bf16, tag="upc",
                                       name=f"upc_{s}_{c}")
                    upc_tiles[(s, c)] = wt
                if fine:
                    # startup-critical ordering: round-0 weights first, the
                    # fp8 tile in quarters right behind the chunks they feed
                    def q8push(i):
                        nc.sync.dma_start(
                            up8_tiles[s][:, :, 1024 * i:1024 * (i + 1)],
                            up8_in[s, :, :, 1024 * i:1024 * (i + 1)])

                    nc.sync.dma_start(upc_tiles[(s, 0)][:, :, 0:256],
                                      upc_in[s, 0, :, :, 0:256])
                    nc.sync.dma_start(upc_tiles[(s, 0)][:, :, 256:512],
                                      upc_in[s, 0, :, :, 256:512])
                    q8push(0)
                    nc.sync.dma_start(upc_tiles[(s, 1)][:], upc_in[s, 1])
                    q8push(1)
                    nc.sync.dma_start(upc_tiles[(s, 2)][:], upc_in[s, 2])
                    nc.sync.dma_start(upc_tiles[(s, 3)][:], upc_in[s, 3])
                    q8push(2)
                    nc.sync.dma_start(upc_tiles[(s, 4)][:], upc_in[s, 4])
                    nc.sync.dma_start(upc_tiles[(s, 5)][:], upc_in[s, 5])
                    q8push(3)
                    nc.sync.dma_start(upc_tiles[(s, 6)][:], upc_in[s, 6])
                    nc.sync.dma_start(upc_tiles[(s, 7)][:], upc_in[s, 7])
                else:
                    nc.sync.dma_start(w8[:], up8_in[s])
                    for c in range(NCH):
                        nc.sync.dma_start(upc_tiles[(s, c)][:], upc_in[s, c])

            # initial DMA pushes (program order = sync-engine issue order;
            # all transfers share one ~400GB/s queue, so order = priority):
            # seg0 first tokens -> seg0 up weights -> seg0 down -> the rest.
            # first pushes go out on idle engines in parallel so the sync
            # engine's ~0.65us-per-descriptor serialization doesn't delay
            # the startup-critical transfers
            c0 = min(512, comp_tok[0])
            nc.sync.dma_start(xnb_t[:, :, 0:c0], xnb_in[:, :, 0:c0])
            nc.sync.dma_start(xn8_t[:, :, 0:c0], xn8_in[:, :, 0:c0])
            push_seg_up(0, fine=True)
            dnb0 = dnb_pool.tile([P, KHB, 1024], bf16, tag="dnb", name="dnb_0")
            nc.sync.dma_start(dnb0[:], dnb_in[0])
            dnb_tiles = {0: dnb0}
            d8 = dn8_pool.tile([P, 2 * N8D, 1024], fp8, tag="dn8", name="dn8_0")
            nc.sync.dma_start(d8[:], dn8_in[0])
            dn8_tiles = {0: d8}
            if c0 < T:
                nc.sync.dma_start(xnb_t[:, :, c0:T], xnb_in[:, :, c0:T])
                nc.sync.dma_start(xn8_t[:, :, c0:T], xn8_in[:, :, c0:T])

            col = 0
            for s in range(K):
                # down bf16 weights: bufs=1, pushed here (s>0) so the WAR
                # wait on the previous segment's last down matmul is met.
                if s > 0:
                    dnbt = dnb_pool.tile([P, KHB, 1024], bf16, tag="dnb",
                                         name=f"dnb_{s}")
                    nc.sync.dma_start(dnbt[:], dnb_in[s])
                    dnb_tiles[s] = dnbt
                dnbt = dnb_tiles[s]

                stok = comp_tok[s]
                rem = stok
                segoff = 0
                while rem > 0:
                    gn = min(512, rem)
                    rem -= gn
                    is_last_group = rem == 0
                    xslc = slice(col + segoff, col + segoff + gn)
                    segoff += gn
                    act_t = act_pool.tile([P, 16, gn], bf16, tag="act")
                    act8_t = act8_pool.tile([P, 2 * N8D, gn], fp8, tag="act8")
                    # ---- up projection: 16 rounds of (a, g) pairs ----
                    for j in range(16):
                        ch = upc_tiles[(s, j // 2)]
                        off = (j % 2) * 256
                        pa = ps.tile([P, gn], f32, tag="ps", name="pa")
                        pg = ps.tile([P, gn], f32, tag="ps", name="pg")
                        for ko in range(KOB):
                            first = ko == 0
                            last = N8U == 0 and ko == KOB - 1
                            nc.tensor.matmul(pa[:], ch[:, ko, off:off + P],
                                             xnb_t[:, ko, xslc],
                                             start=first, stop=last)
                            nc.tensor.matmul(pg[:],
                                             ch[:, ko, off + P:off + 256],
                                             xnb_t[:, ko, xslc],
                                             start=first, stop=last)
                        w8 = up8_tiles[s]
                        for pr in range(N8U):
                            first = KOB == 0 and pr == 0
                            last = pr == N8U - 1
                            kk = slice(2 * pr, 2 * pr + 2)
                            ca = j * 256
                            nc.tensor.matmul(pa[:], w8[:, kk, ca:ca + P],
                                             xn8_t[:, kk, xslc],
                                             start=first, stop=last,
                                             perf_mode=DR)
                            nc.tensor.matmul(pg[:],
                                             w8[:, kk, ca + P:ca + 256],
                                             xn8_t[:, kk, xslc],
                                             start=first, stop=last,
                                             perf_mode=DR)
                        # swiglu: act = pa * silu(pg); fp8 kh's go to act8
                        nc.scalar.activation(act_t[:, j, :], pg[:], Silu)
                        if j < KHB:
                            nc.vector.tensor_mul(act_t[:, j, :], pa[:],
                                                 act_t[:, j, :])
                        else:
                            nc.vector.tensor_mul(act8_t[:, j - KHB, :], pa[:],
                                                 act_t[:, j, :])
                        # prefetch next segment's up weights as their chunk
                        # buffers free up (after this group's last use)
                        if is_last_group and s + 1 < K and j == 15:
                            push_seg_up(s + 1)
                            d8n = dn8_pool.tile([P, 2 * N8D, 1024], fp8,
                                                tag="dn8", name=f"dn8_{s+1}")
                            nc.sync.dma_start(d8n[:], dn8_in[s + 1])
                            dn8_tiles[s + 1] = d8n
                    # ---- down projection: 4 rounds x 2 dout tiles ----
                    d8t = dn8_tiles[s]
                    for rr in range(4):
                        pd = [ps.tile([P, gn], f32, tag="ps", name=f"pd{q}")
                              for q in range(2)]
                        for kh in range(KHB):
                            first = kh == 0
                            last = N8D == 0 and kh == KHB - 1
                            for q in range(2):
                                cc = (2 * rr + q) * P
                                nc.tensor.matmul(pd[q][:],
                                                 dnbt[:, kh, cc:cc + P],
                                                 act_t[:, kh, :],
                                                 start=first, stop=last)
                        for pr in range(N8D):
                            first = KHB == 0 and pr == 0
                            last = pr == N8D - 1
                            kk = slice(2 * pr, 2 * pr + 2)
                            for q in range(2):
                                cc = (2 * rr + q) * P
                                nc.tensor.matmul(pd[q][:],
                                                 d8t[:, kk, cc:cc + P],
                                                 act8_t[:, kk, :],
                                                 start=first, stop=last,
                                                 perf_mode=DR)
                        yc = yc_pool.tile([P, 2, gn], f32, tag="yc")
                        if s == K - 1 and is_last_group and rr == 3:
                            # final round: split the copy across DVE + ACT
                            # and DMA per half so the drain tail is short
                            nc.vector.tensor_copy(yc[:, 0, :], pd[0][:])
                            nc.sync.dma_start(yt_out[:, 6, xslc], yc[:, 0, :])
                            nc.scalar.activation(
                                yc[:, 1, :], pd[1][:],
                                mybir.ActivationFunctionType.Copy)
                            nc.sync.dma_start(yt_out[:, 7, xslc], yc[:, 1, :])
                        else:
                            for q in range(2):
                                nc.vector.tensor_copy(yc[:, q, :], pd[q][:])
                            nc.sync.dma_start(
                                yt_out[:, 2 * rr:2 * rr + 2, xslc], yc[:])
                col += stok

    _patch_bass_json(nc)
    return nc


# ----------------------------------------------------------------------------
# Host-side weight packing
# ----------------------------------------------------------------------------
def _pack_up(up_e):
    """up_w[e] [DIM, 2H] f32 -> (upc [8, P, KOB, 512] bf16,
    up8 [P, 2*N8U, 4096] fp8)."""
    Wd = np.empty((DIM, 4096), dtype=np.float32)
    for j in range(16):
        a = up_e[:, j * P:(j + 1) * P]
        if j >= KHB:
            a = a * (1.0 / SD)
        Wd[:, j * 256:j * 256 + P] = a
        Wd[:, j * 256 + P:(j + 1) * 256] = up_e[:, HID + j * P:HID + (j + 1) * P]
    ub = Wd[:KOB * P].astype(BF16).reshape(KOB, P, NCHU, 512)
    upc = np.ascontiguousarray(ub.transpose(2, 1, 0, 3))
    u8 = _q8(Wd[KOB * P:] * SU).reshape(2 * N8U, P, 4096)
    up8 = np.ascontiguousarray(u8.transpose(1, 0, 2))
    return upc, up8


NCHU = 8


def _pack_dn(dn_e):
    """down_w[e] [HID, DIM] f32 -> (dnb [P, KHB, 1024] bf16,
    dn8 [P, 2*N8D, 1024] fp8)."""
    db = dn_e[:KHB * P].astype(BF16).reshape(KHB, P, DIM)
    dnb = np.ascontiguousarray(db.transpose(1, 0, 2))
    d8 = _q8(dn_e[KHB * P:] * SD).reshape(2 * N8D, P, DIM)
    dn8 = np.ascontiguousarray(d8.transpose(1, 0, 2))
    return dnb, dn8


# ----------------------------------------------------------------------------
# Entry point
# ----------------------------------------------------------------------------
def _run(inputs, trace=False, tmpdir=None):
    from concourse.bass_utils import run_bass_kernel_spmd

    x = np.asarray(inputs["x"])
    scale = np.asarray(inputs["scale"])
    centroids = np.asarray(inputs["centroids"])
    up_w = np.asarray(inputs["up_w"])
    down_w = np.asarray(inputs["down_w"])

    B, S, D = x.shape
    ntok = B * S
    xf32 = x.reshape(ntok, D).astype(np.float32)

    xn, ids = _route(x, scale, centroids)
    comp_tok, slot_expert, chunks, tok_by_e = _plan(ids)
    K = len(comp_tok)
    T = sum(comp_tok)

    up_packed = {}
    dn_packed = {}
    for e in range(N_EXPERTS):
        if any(ee == e for ee in slot_expert.values()):
            up_packed[e] = _pack_up(up_w[e].astype(np.float32))
            dn_packed[e] = _pack_dn(down_w[e].astype(np.float32))

    xnT = np.ascontiguousarray(xn.T)  # [DIM, ntok] f32
    cursor = [0] * N_EXPERTS
    core_cols_tok = [np.zeros(T, dtype=np.int64) for _ in range(N_CORES)]
    core_cols_valid = [np.zeros(T, dtype=bool) for _ in range(N_CORES)]
    in_maps = []
    # fill order must match _plan's chunk assignment (capacity desc, core asc)
    fill_order = {}
    for e in range(N_EXPERTS):
        slots = [(c, j) for (c, j), ee in slot_expert.items() if ee == e]
        slots.sort(key=lambda s: (-comp_tok[s[1]], s[0]))
        fill_order[e] = slots
    seg_start = np.concatenate([[0], np.cumsum(comp_tok)])
    for e in range(N_EXPERTS):
        for (c, j) in fill_order[e]:
            take = chunks[(c, j)]
            if take:
                sel = tok_by_e[e][cursor[e]:cursor[e] + take]
                cursor[e] += take
                a = int(seg_start[j])
                core_cols_tok[c][a:a + take] = sel
                core_cols_valid[c][a:a + take] = True

    for c in range(N_CORES):
        upc = np.zeros((K, NCHU, P, KOB, 512), dtype=BF16)
        up8 = np.zeros((K, P, 2 * N8U, 4096), dtype=FP8)
        dnb = np.zeros((K, P, KHB, 1024), dtype=BF16)
        dn8 = np.zeros((K, P, 2 * N8D, 1024), dtype=FP8)
        for j in range(K):
            e = slot_expert[(c, j)]
            if e is not None:
                upc[j], up8[j] = up_packed[e]
                dnb[j], dn8[j] = dn_packed[e]
        xcols = xnT[:, core_cols_tok[c]]  # [DIM, T] f32 (invalid cols garbage)
        xcols = xcols * core_cols_valid[c][None, :]
        xnb = np.ascontiguousarray(
            xcols[:KOB * P].astype(BF16).reshape(KOB, P, T).transpose(1, 0, 2))
        xn8 = np.ascontiguousarray(
            _q8(xcols[KOB * P:] * (1.0 / SU)).reshape(2 * N8U, P, T)
            .transpose(1, 0, 2))
        in_maps.append({"xnb": xnb, "xn8": xn8, "upc": upc, "up8": up8,
                        "dnb": dnb, "dn8": dn8})

    nc = _build_program(comp_tok)
    kwargs = {}
    if trace:
        kwargs = dict(trace=True, tmpdir=tmpdir)
    res = run_bass_kernel_spmd(nc, in_maps, core_ids=list(range(N_CORES)),
                               **kwargs)

    # ---- scatter + skip ----
    out = xf32.copy()
    for c in range(N_CORES):
        yt = np.ascontiguousarray(
            res.results[c]["yt"].reshape(P, 8, T).transpose(1, 0, 2)
        ).reshape(8 * P, T)  # [DIM, T]
        valid = core_cols_valid[c]
        toks = core_cols_tok[c][valid]
        out[toks] = xf32[toks] + yt[:, valid].T
    return out.reshape(B, S, D).astype(x.dtype), res


def kernel(**inputs) -> np.ndarray:
    out, _ = _run(inputs)
    return out


# revision 26
# speedup vs baseline: 1.2274x; 1.0007x over previous
"""MoE feed-forward (8 experts, hard argmin routing) on 8 TRN2 NeuronCores.

Strategy
--------
Host (numpy): rms_norm + argmin routing, then a dispatch plan at 32-token
granularity: tokens sorted by expert, packed into a UNIFORM per-core
structure of K expert-segments (same sizes on every core; only the data
-- which expert's weights, which tokens -- differs per core).  An exact
cover search (DP over experts) minimizes the per-core token-slot count T.

Numerics: bf16 weights/activations (fp32 PSUM accumulate) for most of the
contraction, with a configurable slice of the contraction computed in
fp8(e4m3) using DoubleRow matmuls (2 k-tiles per instruction -> 2x PE
throughput on that slice, measured 221ns for K=256 N=512 vs 222ns bf16
K=128).  fp8 operands are pre-scaled by powers of 2 (exact) to dodge
e4m3's tiny subnormal range; the down-proj's act scale is folded into the
up-proj "a" weight columns so the device applies it for free.

Device (Bass/Tile, SPMD x8): per segment, weights stream through SBUF in
512-column chunks, each feeding matmul rounds right after it lands.
up-proj -> swiglu (ACT Silu + DVE mul, fp8 slice written as e4m3) ->
down-proj, yT written back to DRAM in fp32 per 2-dout-tile round.

Host: scatter y back to token order and add the skip connection.
"""

import json

import ml_dtypes
import numpy as np

N_EXPERTS = 8
DIM = 1024
HID = 2048
N_CORES = 8
P = 128
EPS = 1e-6
G = 32          # token granularity of the dispatch plan

# fp8 config: number of DoubleRow pairs on each matmul's contraction.
N8U = 1         # up:   N8U pairs of ko-tiles (of 4 pairs = 1024 contraction)
N8D = 1         # down: N8D pairs of kh-tiles (of 8 pairs = 2048 contraction)
SU = 16.0       # up fp8 weight pre-scale (xn fp8 slice divided by SU)
SD = 8.0        # down fp8 weight pre-scale (act fp8 slice divided by SD)
KOB = 8 - 2 * N8U    # bf16 ko-tiles (up)
KHB = 16 - 2 * N8D   # bf16 kh-tiles (down)
WARMUP = 36

BF16 = ml_dtypes.bfloat16
FP8 = ml_dtypes.float8_e4m3fn


def _q8(v):
    return np.clip(v, -240.0, 240.0).astype(FP8)


# ----------------------------------------------------------------------------
# BIR fixup: walrus in this container accepts at most ONE sync-wait per
# instruction.  Split instructions with k>1 waits into (k-1) pure-wait
# EventSemaphore instructions on the same engine immediately before.
# ----------------------------------------------------------------------------
def _split_multiwait_json(bir_bytes: bytes) -> bytes:
    m = json.loads(bir_bytes)
    ctr = 0
    for func in m["functions"]:
        for bb in func["blocks"]:
            out = []
            for inst in bb["instructions"]:
                si = inst.get("sync_info")
                waits = (si or {}).get("on_wait") or []
                if len(waits) > 1:
                    for w in waits[:-1]:
                        ctr += 1
                        out.append({
                            "debug": inst.get("debug", 0),
                            "engine": inst["engine"],
                            "ins": [],
                            "outs": [],
                            "name": f"waitfix_{ctr}",
                            "opcode": "EventSemaphore",
                            "sync_info": {"on_update": [], "on_wait": [w]},
                        })
                    si["on_wait"] = [waits[-1]]
                out.append(inst)
            bb["instructions"] = out
    return json.dumps(m).encode()


def _patch_bass_json(nc):
    orig = nc.to_json_bytes

    def patched():
        return _split_multiwait_json(orig())

    nc.to_json_bytes = patched


# ----------------------------------------------------------------------------
# Host-side routing (replicates the reference numerics in fp32)
# ----------------------------------------------------------------------------
def _route(x, scale, centroids):
    xf = x.reshape(-1, DIM).astype(np.float32)
    ms = np.mean(xf * xf, axis=-1, keepdims=True)
    s = scale.astype(np.float32) / np.sqrt(ms + EPS)
    xn = xf * s
    nx = np.sum(xn * xn, axis=-1)[:, None]
    ny = np.sum(centroids * centroids, axis=-1)[None, :]
    d2 = nx + ny - 2.0 * (xn @ centroids.T)
    ids = np.argmin(d2, axis=-1).astype(np.int32)
    return xn, ids


# ----------------------------------------------------------------------------
# Dispatch planner: uniform comp across cores, exact cover, 32-token units
# ----------------------------------------------------------------------------
def _compositions(total, k):
    if k == 1:
        yield (total,)
        return
    for first in range((total + k - 1) // k, total - k + 2):
        for rest in _compositions(total - first, k - 1):
            if rest[0] <= first:
                yield (first,) + rest


def _cover(comp, units):
    """comp: slot sizes (units), 8 slots each. Returns {e: counts per pos}."""
    K = len(comp)
    experts = sorted(range(len(units)), key=lambda e: -units[e])
    avail = [N_CORES] * K

    def rec(i):
        if i == len(experts):
            return {}
        e = experts[i]
        need = units[e]
        if need == 0:
            rest = rec(i + 1)
            if rest is not None:
                rest[e] = (0,) * K
            return rest
        opts = []
        max_counts = [min(avail[j], (need + comp[j] - 1) // comp[j])
                      for j in range(K)]

        def enum(j, counts, cap):
            if cap >= need:
                if all(c == 0 or cap - comp[k2] < need
                       for k2, c in enumerate(counts)):
                    opts.append((cap - need, tuple(counts)))
                return
            if j == K:
                return
            for c in range(max_counts[j] + 1):
                counts[j] = c
                enum(j + 1, counts, cap + c * comp[j])
                if cap + c * comp[j] >= need:
                    break
            counts[j] = 0

        enum(0, [0] * K, 0)
        opts.sort()
        for _, counts in opts[:60]:
            for j in range(K):
                avail[j] -= counts[j]
            rest = rec(i + 1)
            for j in range(K):
                avail[j] += counts[j]
            if rest is not None:
                rest[e] = counts
                return rest
        return None

    return rec(0)


def _seg_cost(gn):
    """PE-time model (ns) for one token group of gn tokens."""
    n_up = KOB * 2 + N8U * 2
    ldw_up = KOB * 2 * 107 + N8U * 2 * 214
    n_dn = KHB * 2 + N8D * 2
    ldw_dn = KHB * 2 * 107 + N8D * 2 * 214
    up = 16 * max(n_up * (gn / 2.4 + 2.5), ldw_up)
    dn = 4 * max(n_dn * (gn / 2.4 + 2.5), ldw_dn)
    return up + dn


def _comp_cost(comp_units):
    c = 0.0
    for u in comp_units:
        stok = u * G
        while stok > 0:
            gn = min(512, stok)
            stok -= gn
            c += _seg_cost(gn)
    return c


def _plan(ids):
    cnt = np.bincount(ids, minlength=N_EXPERTS)
    units = [int((c + G - 1) // G) for c in cnt]
    total = sum(units)
    lo = (total + N_CORES - 1) // N_CORES
    found = None
    for T in range(lo, lo + 40):
        cands = []
        for K in (2, 3, 4):
            cands += list(_compositions(T, K))
        # try cheapest comps first (cost model includes a per-segment
        # penalty for the extra weight DMA); first feasible wins
        cands.sort(key=lambda comp: _comp_cost(comp) + 2000.0 * len(comp))
        for comp in cands:
            sol = _cover(comp, units)
            if sol is not None:
                found = (comp, sol)
                break
        if found:
            break
    if not found:
        raise RuntimeError("dispatch packing failed")
    comp, sol = found
    # largest segment first: its down phase starts latest, giving the
    # (bandwidth-limited) startup DMA time to deliver the down weights;
    # smallest last so the final output-DMA tail is short.
    order = np.argsort([-c for c in comp], kind="stable")
    comp = tuple(comp[j] for j in order)
    sol = {e: tuple(s[j] for j in order) for e, s in sol.items()}
    # slots: position j -> list of experts (len 8, None = unused)
    tok_by_e = [np.where(ids == e)[0] for e in range(N_EXPERTS)]
    slot_expert = {}
    for j in range(len(comp)):
        lst = []
        for e in range(N_EXPERTS):
            lst += [e] * sol[e][j]
        assert len(lst) <= N_CORES
        lst += [None] * (N_CORES - len(lst))
        for c in range(N_CORES):
            slot_expert[(c, j)] = lst[c]
    # fill tokens: per expert, slots ordered by capacity desc
    comp_tok = tuple(c * G for c in comp)
    cursor = [0] * N_EXPERTS
    chunks = {}
    for e in range(N_EXPERTS):
        slots = [(c, j) for (c, j), ee in slot_expert.items() if ee == e]
        slots.sort(key=lambda s: (-comp_tok[s[1]], s[0]))
        for (c, j) in slots:
            take = min(comp_tok[j], len(tok_by_e[e]) - cursor[e])
            take = max(take, 0)
            chunks[(c, j)] = take
            cursor[e] += take
    for e in range(N_EXPERTS):
        assert cursor[e] == len(tok_by_e[e]), "plan did not cover all tokens"
    return comp_tok, slot_expert, chunks, tok_by_e


# ----------------------------------------------------------------------------
# Device program
# ----------------------------------------------------------------------------
def _build_program(comp_tok):
    import concourse.bass as bass
    import concourse.mybir as mybir
    import concourse.tile as tile

    f32 = mybir.dt.float32
    bf16 = mybir.dt.bfloat16
    fp8 = mybir.dt.float8e4
    Silu = mybir.ActivationFunctionType.Silu
    DR = mybir.MatmulPerfMode.DoubleRow

    K = len(comp_tok)
    T = sum(comp_tok)
    NCH = 8  # 512-col chunks of the 4096 up output dim

    nc = bass.Bass("TRN2", debug=False)
    xnb_in = nc.dram_tensor("xnb", [P, KOB, T], bf16, kind="ExternalInput").ap()
    xn8_in = nc.dram_tensor("xn8", [P, 2 * N8U, T], fp8,
                            kind="ExternalInput").ap()
    upc_in = nc.dram_tensor("upc", [K, NCH, P, KOB, 512], bf16,
                            kind="ExternalInput").ap()
    up8_in = nc.dram_tensor("up8", [K, P, 2 * N8U, 4096], fp8,
                            kind="ExternalInput").ap()
    dnb_in = nc.dram_tensor("dnb", [K, P, KHB, 1024], bf16,
                            kind="ExternalInput").ap()
    dn8_in = nc.dram_tensor("dn8", [K, P, 2 * N8D, 1024], fp8,
                            kind="ExternalInput").ap()
    yt_out = nc.dram_tensor("yt", [P, 8, T], f32, kind="ExternalOutput").ap()

    with tile.TileContext(nc) as tc:
        with (
            tc.tile_pool(name="upc", bufs=12) as upc_pool,
            tc.tile_pool(name="up8", bufs=2) as up8_pool,
            tc.tile_pool(name="dnb", bufs=1) as dnb_pool,
            tc.tile_pool(name="dn8", bufs=2) as dn8_pool,
            tc.tile_pool(name="xnb", bufs=1) as xnb_pool,
            tc.tile_pool(name="xn8", bufs=1) as xn8_pool,
            tc.tile_pool(name="act", bufs=2) as act_pool,
            tc.tile_pool(name="act8", bufs=2) as act8_pool,
            tc.tile_pool(name="yc", bufs=4) as yc_pool,
            tc.tile_pool(name="warm", bufs=1) as warm_pool,
            tc.tile_pool(name="ps", bufs=8, space="PSUM") as ps,
        ):
            # PE warm-up: dependency-free matmuls on a zeroed scratch tile
            # keep PE busy while the first DMAs land.
            wsrc = warm_pool.tile([P, 256], bf16, tag="warm")
            nc.gpsimd.memset(wsrc[:], 0.0)
            wps = [ps.tile([P, P], f32, tag="ps", name=f"wps{i}")
                   for i in range(2)]
            for i in range(WARMUP):
                nc.tensor.matmul(wps[i % 2][:], wsrc[:, 0:P],
                                 wsrc[:, P:2 * P], start=True, stop=True)

            xnb_t = xnb_pool.tile([P, KOB, T], bf16, tag="xnb")
            xn8_t = xn8_pool.tile([P, 2 * N8U, T], fp8, tag="xn8")

            up8_tiles = {}
            upc_tiles = {}

            def push_seg_up(s, fine=False):
                w8 = up8_pool.tile([P, 2 * N8U, 4096], fp8, tag="up8",
                                   name=f"up8_{s}")
                up8_tiles[s] = w8
                for c in range(NCH):
                    wt = upc_pool.tile([P, KOB, 512], bf16, tag="upc",
                                       name=f"upc_{s}_{c}")
                    upc_tiles[(s, c)] = wt
                if fine:
                    # startup-critical ordering: push in consumption order,
                    # with tiny SBUF-read "gate" DMAs in between -- each
                    # stalls the sync queue until an earlier transfer lands,
                    # bounding how many streams share the HBM bandwidth.
                    def q8push(i):
                        nc.sync.dma_start(
                            up8_tiles[s][:, :, 1024 * i:1024 * (i + 1)],
                            up8_in[s, :, :, 1024 * i:1024 * (i + 1)])

                    ngate = [0]

                    def gate(src):
                        i = ngate[0]
                        ngate[0] += 1
                        nc.sync.dma_start(dly[:, 64 * i:64 * (i + 1)], src)

                    ch = upc_tiles
                    nc.sync.dma_start(ch[(s, 0)][:, :, 0:256],
                                      upc_in[s, 0, :, :, 0:256])
                    gate(xnb_t[:, 0, 0:64])
                    q8push(0)
                    nc.sync.dma_start(ch[(s, 0)][:, :, 256:512],
                                      upc_in[s, 0, :, :, 256:512])
                    gate(ch[(s, 0)][:, 0, 0:64])
                    nc.sync.dma_start(ch[(s, 1)][:], upc_in[s, 1])
                    gate(ch[(s, 0)][:, 0, 256:320])
                    q8push(1)
                    nc.sync.dma_start(ch[(s, 2)][:], upc_in[s, 2])
                    gate(ch[(s, 1)][:, 0, 0:64])
                    nc.sync.dma_start(ch[(s, 3)][:], upc_in[s, 3])
                    gate(ch[(s, 2)][:, 0, 0:64])
                    q8push(2)
                    nc.sync.dma_start(ch[(s, 4)][:], upc_in[s, 4])
                    gate(ch[(s, 3)][:, 0, 0:64])
                    nc.sync.dma_start(ch[(s, 5)][:], upc_in[s, 5])
                    gate(ch[(s, 4)][:, 0, 0:64])
                    q8push(3)
                    nc.sync.dma_start(ch[(s, 6)][:], upc_in[s, 6])
                    gate(ch[(s, 5)][:, 0, 0:64])
                    nc.sync.dma_start(ch[(s, 7)][:], upc_in[s, 7])
                else:
                    nc.sync.dma_start(w8[:], up8_in[s])
                    for c in range(NCH):
                        nc.sync.dma_start(upc_tiles[(s, c)][:], upc_in[s, c])

            # initial DMA pushes (program order = sync-engine issue order;
            # all transfers share one ~400GB/s queue, so order = priority):
            # seg0 first tokens -> seg0 up weights -> seg0 down -> the rest.
            # first pushes go out on idle engines in parallel so the sync
            # engine's ~0.65us-per-descriptor serialization doesn't delay
            # the startup-critical transfers
            c0 = min(512, comp_tok[0])
            dly = warm_pool.tile([P, 64 * 12], bf16, tag="dly")
            nc.sync.dma_start(xnb_t[:, :, 0:c0], xnb_in[:, :, 0:c0])
            nc.sync.dma_start(xn8_t[:, :, 0:c0], xn8_in[:, :, 0:c0])
            push_seg_up(0, fine=True)
            dnb0 = dnb_pool.tile([P, KHB, 1024], bf16, tag="dnb", name="dnb_0")
            nc.sync.dma_start(dnb0[:], dnb_in[0])
            dnb_tiles = {0: dnb0}
            d8 = dn8_pool.tile([P, 2 * N8D, 1024], fp8, tag="dn8", name="dn8_0")
            nc.sync.dma_start(d8[:], dn8_in[0])
            dn8_tiles = {0: d8}
            if c0 < T:
                nc.sync.dma_start(xnb_t[:, :, c0:T], xnb_in[:, :, c0:T])
                nc.sync.dma_start(xn8_t[:, :, c0:T], xn8_in[:, :, c0:T])

            col = 0
            for s in range(K):
                # down bf16 weights: bufs=1, pushed here (s>0) so the WAR
                # wait on the previous segment's last down matmul is met.
                if s > 0:
                    dnbt = dnb_pool.tile([P, KHB, 1024], bf16, tag="dnb",
                                         name=f"dnb_{s}")
                    nc.sync.dma_start(dnbt[:], dnb_in[s])
                    dnb_tiles[s] = dnbt
                dnbt = dnb_tiles[s]

                stok = comp_tok[s]
                rem = stok
                segoff = 0
                while rem > 0:
                    gn = min(512, rem)
                    rem -= gn
                    is_last_group = rem == 0
                    xslc = slice(col + segoff, col + segoff + gn)
                    segoff += gn
                    act_t = act_pool.tile([P, 16, gn], bf16, tag="act")
                    act8_t = act8_pool.tile([P, 2 * N8D, gn], fp8, tag="act8")
                    # ---- up projection: 16 rounds of (a, g) pairs ----
                    for j in range(16):
                        ch = upc_tiles[(s, j // 2)]
                        off = (j % 2) * 256
                        pa = ps.tile([P, gn], f32, tag="ps", name="pa")
                        pg = ps.tile([P, gn], f32, tag="ps", name="pg")
                        for ko in range(KOB):
                            first = ko == 0
                            last = N8U == 0 and ko == KOB - 1
                            nc.tensor.matmul(pa[:], ch[:, ko, off:off + P],
                                             xnb_t[:, ko, xslc],
                                             start=first, stop=last)
                            nc.tensor.matmul(pg[:],
                                             ch[:, ko, off + P:off + 256],
                                             xnb_t[:, ko, xslc],
                                             start=first, stop=last)
                        w8 = up8_tiles[s]
                        for pr in range(N8U):
                            first = KOB == 0 and pr == 0
                            last = pr == N8U - 1
                            kk = slice(2 * pr, 2 * pr + 2)
                            ca = j * 256
                            nc.tensor.matmul(pa[:], w8[:, kk, ca:ca + P],
                                             xn8_t[:, kk, xslc],
                                             start=first, stop=last,
                                             perf_mode=DR)
                            nc.tensor.matmul(pg[:],
                                             w8[:, kk, ca + P:ca + 256],
                                             xn8_t[:, kk, xslc],
                                             start=first, stop=last,
                                             perf_mode=DR)
                        # swiglu: act = pa * silu(pg); fp8 kh's go to act8
                        nc.scalar.activation(act_t[:, j, :], pg[:], Silu)
                        if j < KHB:
                            nc.vector.tensor_mul(act_t[:, j, :], pa[:],
                                                 act_t[:, j, :])
                        else:
                            nc.vector.tensor_mul(act8_t[:, j - KHB, :], pa[:],
                                                 act_t[:, j, :])
                        # prefetch next segment's up weights as their chunk
                        # buffers free up (after this group's last use)
                        if is_last_group and s + 1 < K and j == 15:
                            push_seg_up(s + 1)
                            d8n = dn8_pool.tile([P, 2 * N8D, 1024], fp8,
                                                tag="dn8", name=f"dn8_{s+1}")
                            nc.sync.dma_start(d8n[:], dn8_in[s + 1])
                            dn8_tiles[s + 1] = d8n
                    # ---- down projection: 4 rounds x 2 dout tiles ----
                    d8t = dn8_tiles[s]
                    for rr in range(4):
                        pd = [ps.tile([P, gn], f32, tag="ps", name=f"pd{q}")
                              for q in range(2)]
                        for kh in range(KHB):
                            first = kh == 0
                            last = N8D == 0 and kh == KHB - 1
                            for q in range(2):
                                cc = (2 * rr + q) * P
                                nc.tensor.matmul(pd[q][:],
                                                 dnbt[:, kh, cc:cc + P],
                                                 act_t[:, kh, :],
                                                 start=first, stop=last)
                        for pr in range(N8D):
                            first = KHB == 0 and pr == 0
                            last = pr == N8D - 1
                            kk = slice(2 * pr, 2 * pr + 2)
                            for q in range(2):
                                cc = (2 * rr + q) * P
                                nc.tensor.matmul(pd[q][:],
                                                 d8t[:, kk, cc:cc + P],
                                                 act8_t[:, kk, :],
                                                 start=first, stop=last,
                                                 perf_mode=DR)
                        yc = yc_pool.tile([P, 2, gn], f32, tag="yc")
                        if s == K - 1 and is_last_group and rr == 3:
                            # final round: split the copy across DVE + ACT
                            # and DMA per half so the drain tail is short
                            nc.vector.tensor_copy(yc[:, 0, :], pd[0][:])
                            nc.sync.dma_start(yt_out[:, 6, xslc], yc[:, 0, :])
                            nc.scalar.activation(
                                yc[:, 1, :], pd[1][:],
                                mybir.ActivationFunctionType.Copy)
                            nc.sync.dma_start(yt_out[:, 7, xslc], yc[:, 1, :])
                        else:
                            for q in range(2):
                                nc.vector.tensor_copy(yc[:, q, :], pd[q][:])
                            nc.sync.dma_start(
                                yt_out[:, 2 * rr:2 * rr + 2, xslc], yc[:])
                col += stok

    _patch_bass_json(nc)
    return nc


# ----------------------------------------------------------------------------
# Host-side weight packing
# ----------------------------------------------------------------------------
def _pack_up(up_e):
    """up_w[e] [DIM, 2H] f32 -> (upc [8, P, KOB, 512] bf16,
    up8 [P, 2*N8U, 4096] fp8)."""
    Wd = np.empty((DIM, 4096), dtype=np.float32)
    for j in range(16):
        a = up_e[:, j * P:(j + 1) * P]
        if j >= KHB:
            a = a * (1.0 / SD)
        Wd[:, j * 256:j * 256 + P] = a
        Wd[:, j * 256 + P:(j + 1) * 256] = up_e[:, HID + j * P:HID + (j + 1) * P]
    ub = Wd[:KOB * P].astype(BF16).reshape(KOB, P, NCHU, 512)
    upc = np.ascontiguousarray(ub.transpose(2, 1, 0, 3))
    u8 = _q8(Wd[KOB * P:] * SU).reshape(2 * N8U, P, 4096)
    up8 = np.ascontiguousarray(u8.transpose(1, 0, 2))
    return upc, up8


NCHU = 8


def _pack_dn(dn_e):
    """down_w[e] [HID, DIM] f32 -> (dnb [P, KHB, 1024] bf16,
    dn8 [P, 2*N8D, 1024] fp8)."""
    db = dn_e[:KHB * P].astype(BF16).reshape(KHB, P, DIM)
    dnb = np.ascontiguousarray(db.transpose(1, 0, 2))
    d8 = _q8(dn_e[KHB * P:] * SD).reshape(2 * N8D, P, DIM)
    dn8 = np.ascontiguousarray(d8.transpose(1, 0, 2))
    return dnb, dn8


# ----------------------------------------------------------------------------
# Entry point
# ----------------------------------------------------------------------------
def _run(inputs, trace=False, tmpdir=None):
    from concourse.bass_utils import run_bass_kernel_spmd

    x = np.asarray(inputs["x"])
    scale = np.asarray(inputs["scale"])
    centroids = np.asarray(inputs["centroids"])
    up_w = np.asarray(inputs["up_w"])
    down_w = np.asarray(inputs["down_w"])

    B, S, D = x.shape
    ntok = B * S
    xf32 = x.reshape(ntok, D).astype(np.float32)

    xn, ids = _route(x, scale, centroids)
    comp_tok, slot_expert, chunks, tok_by_e = _plan(ids)
    K = len(comp_tok)
    T = sum(comp_tok)

    up_packed = {}
    dn_packed = {}
    for e in range(N_EXPERTS):
        if any(ee == e for ee in slot_expert.values()):
            up_packed[e] = _pack_up(up_w[e].astype(np.float32))
            dn_packed[e] = _pack_dn(down_w[e].astype(np.float32))

    xnT = np.ascontiguousarray(xn.T)  # [DIM, ntok] f32
    cursor = [0] * N_EXPERTS
    core_cols_tok = [np.zeros(T, dtype=np.int64) for _ in range(N_CORES)]
    core_cols_valid = [np.zeros(T, dtype=bool) for _ in range(N_CORES)]
    in_maps = []
    # fill order must match _plan's chunk assignment (capacity desc, core asc)
    fill_order = {}
    for e in range(N_EXPERTS):
        slots = [(c, j) for (c, j), ee in slot_expert.items() if ee == e]
        slots.sort(key=lambda s: (-comp_tok[s[1]], s[0]))
        fill_order[e] = slots
    seg_start = np.concatenate([[0], np.cumsum(comp_tok)])
    for e in range(N_EXPERTS):
        for (c, j) in fill_order[e]:
            take = chunks[(c, j)]
            if take:
                sel = tok_by_e[e][cursor[e]:cursor[e] + take]
                cursor[e] += take
                a = int(seg_start[j])
                core_cols_tok[c][a:a + take] = sel
                core_cols_valid[c][a:a + take] = True

    for c in range(N_CORES):
        upc = np.zeros((K, NCHU, P, KOB, 512), dtype=BF16)
        up8 = np.zeros((K, P, 2 * N8U, 4096), dtype=FP8)
        dnb = np.zeros((K, P, KHB, 1024), dtype=BF16)
        dn8 = np.zeros((K, P, 2 * N8D, 1024), dtype=FP8)
        for j in range(K):
            e = slot_expert[(c, j)]
            if e is not None:
                upc[j], up8[j] = up_packed[e]
                dnb[j], dn8[j] = dn_packed[e]
        xcols = xnT[:, core_cols_tok[c]]  # [DIM, T] f32 (invalid cols garbage)
        xcols = xcols * core_cols_valid[c][None, :]
        xnb = np.ascontiguousarray(
            xcols[:KOB * P].astype(BF16).reshape(KOB, P, T).transpose(1, 0, 2))
        xn8 = np.ascontiguousarray(
            _q8(xcols[KOB * P:] * (1.0 / SU)).reshape(2 * N8U, P, T)
            .transpose(1, 0, 2))
        in_maps.append({"xnb": xnb, "xn8": xn8, "upc": upc, "up8": up8,
                        "dnb": dnb, "dn8": dn8})

    nc = _build_program(comp_tok)
    kwargs = {}
    if trace:
        kwargs = dict(trace=True, tmpdir=tmpdir)
    res = run_bass_kernel_spmd(nc, in_maps, core_ids=list(range(N_CORES)),
                               **kwargs)

    # ---- scatter + skip ----
    out = xf32.copy()
    for c in range(N_CORES):
        yt = np.ascontiguousarray(
            res.results[c]["yt"].reshape(P, 8, T).transpose(1, 0, 2)
        ).reshape(8 * P, T)  # [DIM, T]
        valid = core_cols_valid[c]
        toks = core_cols_tok[c][valid]
        out[toks] = xf32[toks] + yt[:, valid].T
    return out.reshape(B, S, D).astype(x.dtype), res


def kernel(**inputs) -> np.ndarray:
    out, _ = _run(inputs)
    return out


# revision 28
# speedup vs baseline: 1.2529x; 1.0208x over previous
"""MoE feed-forward (8 experts, hard argmin routing) on 8 TRN2 NeuronCores.

Strategy
--------
Host (numpy): rms_norm + argmin routing, then a dispatch plan at 32-token
granularity: tokens sorted by expert, packed into a UNIFORM per-core
structure of K expert-segments (same sizes on every core; only the data
-- which expert's weights, which tokens -- differs per core).  An exact
cover search (DP over experts) minimizes the per-core token-slot count T.

Numerics: bf16 weights/activations (fp32 PSUM accumulate) for most of the
contraction, with a configurable slice of the contraction computed in
fp8(e4m3) using DoubleRow matmuls (2 k-tiles per instruction -> 2x PE
throughput on that slice, measured 221ns for K=256 N=512 vs 222ns bf16
K=128).  fp8 operands are pre-scaled by powers of 2 (exact) to dodge
e4m3's tiny subnormal range; the down-proj's act scale is folded into the
up-proj "a" weight columns so the device applies it for free.

Device (Bass/Tile, SPMD x8): per segment, weights stream through SBUF in
512-column chunks, each feeding matmul rounds right after it lands.
up-proj -> swiglu (ACT Silu + DVE mul, fp8 slice written as e4m3) ->
down-proj, yT written back to DRAM in fp32 per 2-dout-tile round.

Host: scatter y back to token order and add the skip connection.
"""

import json

import ml_dtypes
import numpy as np

N_EXPERTS = 8
DIM = 1024
HID = 2048
N_CORES = 8
P = 128
EPS = 1e-6
G = 32          # token granularity of the dispatch plan

# fp8 config: number of DoubleRow pairs on each matmul's contraction.
N8U = 1         # up:   N8U pairs of ko-tiles (of 4 pairs = 1024 contraction)
N8D = 2         # down: N8D pairs of kh-tiles (of 8 pairs = 2048 contraction)
SU = 16.0       # up fp8 weight pre-scale (xn fp8 slice divided by SU)
SD = 8.0        # down fp8 weight pre-scale (act fp8 slice divided by SD)
KOB = 8 - 2 * N8U    # bf16 ko-tiles (up)
KHB = 16 - 2 * N8D   # bf16 kh-tiles (down)
WARMUP = 36

BF16 = ml_dtypes.bfloat16
FP8 = ml_dtypes.float8_e4m3fn


def _q8(v):
    return np.clip(v, -240.0, 240.0).astype(FP8)


# ----------------------------------------------------------------------------
# BIR fixup: walrus in this container accepts at most ONE sync-wait per
# instruction.  Split instructions with k>1 waits into (k-1) pure-wait
# EventSemaphore instructions on the same engine immediately before.
# ----------------------------------------------------------------------------
def _split_multiwait_json(bir_bytes: bytes) -> bytes:
    m = json.loads(bir_bytes)
    ctr = 0
    for func in m["functions"]:
        for bb in func["blocks"]:
            out = []
            for inst in bb["instructions"]:
                si = inst.get("sync_info")
                waits = (si or {}).get("on_wait") or []
                if len(waits) > 1:
                    for w in waits[:-1]:
                        ctr += 1
                        out.append({
                            "debug": inst.get("debug", 0),
                            "engine": inst["engine"],
                            "ins": [],
                            "outs": [],
                            "name": f"waitfix_{ctr}",
                            "opcode": "EventSemaphore",
                            "sync_info": {"on_update": [], "on_wait": [w]},
                        })
                    si["on_wait"] = [waits[-1]]
                out.append(inst)
            bb["instructions"] = out
    return json.dumps(m).encode()


def _patch_bass_json(nc):
    orig = nc.to_json_bytes

    def patched():
        return _split_multiwait_json(orig())

    nc.to_json_bytes = patched


# ----------------------------------------------------------------------------
# Host-side routing (replicates the reference numerics in fp32)
# ----------------------------------------------------------------------------
def _route(x, scale, centroids):
    xf = x.reshape(-1, DIM).astype(np.float32)
    ms = np.mean(xf * xf, axis=-1, keepdims=True)
    s = scale.astype(np.float32) / np.sqrt(ms + EPS)
    xn = xf * s
    nx = np.sum(xn * xn, axis=-1)[:, None]
    ny = np.sum(centroids * centroids, axis=-1)[None, :]
    d2 = nx + ny - 2.0 * (xn @ centroids.T)
    ids = np.argmin(d2, axis=-1).astype(np.int32)
    return xn, ids


# ----------------------------------------------------------------------------
# Dispatch planner: uniform comp across cores, exact cover, 32-token units
# ----------------------------------------------------------------------------
def _compositions(total, k):
    if k == 1:
        yield (total,)
        return
    for first in range((total + k - 1) // k, total - k + 2):
        for rest in _compositions(total - first, k - 1):
            if rest[0] <= first:
                yield (first,) + rest


def _cover(comp, units):
    """comp: slot sizes (units), 8 slots each. Returns {e: counts per pos}."""
    K = len(comp)
    experts = sorted(range(len(units)), key=lambda e: -units[e])
    avail = [N_CORES] * K

    def rec(i):
        if i == len(experts):
            return {}
        e = experts[i]
        need = units[e]
        if need == 0:
            rest = rec(i + 1)
            if rest is not None:
                rest[e] = (0,) * K
            return rest
        opts = []
        max_counts = [min(avail[j], (need + comp[j] - 1) // comp[j])
                      for j in range(K)]

        def enum(j, counts, cap):
            if cap >= need:
                if all(c == 0 or cap - comp[k2] < need
                       for k2, c in enumerate(counts)):
                    opts.append((cap - need, tuple(counts)))
                return
            if j == K:
                return
            for c in range(max_counts[j] + 1):
                counts[j] = c
                enum(j + 1, counts, cap + c * comp[j])
                if cap + c * comp[j] >= need:
                    break
            counts[j] = 0

        enum(0, [0] * K, 0)
        opts.sort()
        for _, counts in opts[:60]:
            for j in range(K):
                avail[j] -= counts[j]
            rest = rec(i + 1)
            for j in range(K):
                avail[j] += counts[j]
            if rest is not None:
                rest[e] = counts
                return rest
        return None

    return rec(0)


def _seg_cost(gn):
    """PE-time model (ns) for one token group of gn tokens."""
    n_up = KOB * 2 + N8U * 2
    ldw_up = KOB * 2 * 107 + N8U * 2 * 214
    n_dn = KHB * 2 + N8D * 2
    ldw_dn = KHB * 2 * 107 + N8D * 2 * 214
    up = 16 * max(n_up * (gn / 2.4 + 2.5), ldw_up)
    dn = 4 * max(n_dn * (gn / 2.4 + 2.5), ldw_dn)
    return up + dn


def _comp_cost(comp_units):
    c = 0.0
    for u in comp_units:
        stok = u * G
        while stok > 0:
            gn = min(512, stok)
            stok -= gn
            c += _seg_cost(gn)
    return c


def _plan(ids):
    cnt = np.bincount(ids, minlength=N_EXPERTS)
    units = [int((c + G - 1) // G) for c in cnt]
    total = sum(units)
    lo = (total + N_CORES - 1) // N_CORES
    found = None
    for T in range(lo, lo + 40):
        cands = []
        for K in (2, 3, 4):
            cands += list(_compositions(T, K))
        # try cheapest comps first (cost model includes a per-segment
        # penalty for the extra weight DMA); first feasible wins
        cands.sort(key=lambda comp: _comp_cost(comp) + 2000.0 * len(comp))
        for comp in cands:
            sol = _cover(comp, units)
            if sol is not None:
                found = (comp, sol)
                break
        if found:
            break
    if not found:
        raise RuntimeError("dispatch packing failed")
    comp, sol = found
    # largest segment first: its down phase starts latest, giving the
    # (bandwidth-limited) startup DMA time to deliver the down weights;
    # smallest last so the final output-DMA tail is short.
    order = np.argsort([-c for c in comp], kind="stable")
    comp = tuple(comp[j] for j in order)
    sol = {e: tuple(s[j] for j in order) for e, s in sol.items()}
    # slots: position j -> list of experts (len 8, None = unused)
    tok_by_e = [np.where(ids == e)[0] for e in range(N_EXPERTS)]
    slot_expert = {}
    for j in range(len(comp)):
        lst = []
        for e in range(N_EXPERTS):
            lst += [e] * sol[e][j]
        assert len(lst) <= N_CORES
        lst += [None] * (N_CORES - len(lst))
        for c in range(N_CORES):
            slot_expert[(c, j)] = lst[c]
    # fill tokens: per expert, slots ordered by capacity desc
    comp_tok = tuple(c * G for c in comp)
    cursor = [0] * N_EXPERTS
    chunks = {}
    for e in range(N_EXPERTS):
        slots = [(c, j) for (c, j), ee in slot_expert.items() if ee == e]
        slots.sort(key=lambda s: (-comp_tok[s[1]], s[0]))
        for (c, j) in slots:
            take = min(comp_tok[j], len(tok_by_e[e]) - cursor[e])
            take = max(take, 0)
            chunks[(c, j)] = take
            cursor[e] += take
    for e in range(N_EXPERTS):
        assert cursor[e] == len(tok_by_e[e]), "plan did not cover all tokens"
    return comp_tok, slot_expert, chunks, tok_by_e


# ----------------------------------------------------------------------------
# Device program
# ----------------------------------------------------------------------------
def _build_program(comp_tok):
    import concourse.bass as bass
    import concourse.mybir as mybir
    import concourse.tile as tile

    f32 = mybir.dt.float32
    bf16 = mybir.dt.bfloat16
    fp8 = mybir.dt.float8e4
    Silu = mybir.ActivationFunctionType.Silu
    DR = mybir.MatmulPerfMode.DoubleRow

    K = len(comp_tok)
    T = sum(comp_tok)
    NCH = 8  # 512-col chunks of the 4096 up output dim

    nc = bass.Bass("TRN2", debug=False)
    xnb_in = nc.dram_tensor("xnb", [P, KOB, T], bf16, kind="ExternalInput").ap()
    xn8_in = nc.dram_tensor("xn8", [P, 2 * N8U, T], fp8,
                            kind="ExternalInput").ap()
    upc_in = nc.dram_tensor("upc", [K, NCH, P, KOB, 512], bf16,
                            kind="ExternalInput").ap()
    up8_in = nc.dram_tensor("up8", [K, P, 2 * N8U, 4096], fp8,
                            kind="ExternalInput").ap()
    dnb_in = nc.dram_tensor("dnb", [K, P, KHB, 1024], bf16,
                            kind="ExternalInput").ap()
    dn8_in = nc.dram_tensor("dn8", [K, P, 2 * N8D, 1024], fp8,
                            kind="ExternalInput").ap()
    yt_out = nc.dram_tensor("yt", [P, 8, T], f32, kind="ExternalOutput").ap()

    with tile.TileContext(nc) as tc:
        with (
            tc.tile_pool(name="upc", bufs=12) as upc_pool,
            tc.tile_pool(name="up8", bufs=2) as up8_pool,
            tc.tile_pool(name="dnb", bufs=1) as dnb_pool,
            tc.tile_pool(name="dn8", bufs=2) as dn8_pool,
            tc.tile_pool(name="xnb", bufs=1) as xnb_pool,
            tc.tile_pool(name="xn8", bufs=1) as xn8_pool,
            tc.tile_pool(name="act", bufs=2) as act_pool,
            tc.tile_pool(name="act8", bufs=2) as act8_pool,
            tc.tile_pool(name="yc", bufs=4) as yc_pool,
            tc.tile_pool(name="warm", bufs=1) as warm_pool,
            tc.tile_pool(name="ps", bufs=8, space="PSUM") as ps,
        ):
            # PE warm-up: dependency-free matmuls on a zeroed scratch tile
            # keep PE busy while the first DMAs land.
            wsrc = warm_pool.tile([P, 256], bf16, tag="warm")
            nc.gpsimd.memset(wsrc[:], 0.0)
            wps = [ps.tile([P, P], f32, tag="ps", name=f"wps{i}")
                   for i in range(2)]
            for i in range(WARMUP):
                nc.tensor.matmul(wps[i % 2][:], wsrc[:, 0:P],
                                 wsrc[:, P:2 * P], start=True, stop=True)

            xnb_t = xnb_pool.tile([P, KOB, T], bf16, tag="xnb")
            xn8_t = xn8_pool.tile([P, 2 * N8U, T], fp8, tag="xn8")

            up8_tiles = {}
            upc_tiles = {}

            def push_seg_up(s, fine=False):
                w8 = up8_pool.tile([P, 2 * N8U, 4096], fp8, tag="up8",
                                   name=f"up8_{s}")
                up8_tiles[s] = w8
                for c in range(NCH):
                    wt = upc_pool.tile([P, KOB, 512], bf16, tag="upc",
                                       name=f"upc_{s}_{c}")
                    upc_tiles[(s, c)] = wt
                if fine:
                    # startup-critical ordering: push in consumption order,
                    # with tiny SBUF-read "gate" DMAs in between -- each
                    # stalls the sync queue until an earlier transfer lands,
                    # bounding how many streams share the HBM bandwidth.
                    def q8push(i):
                        nc.sync.dma_start(
                            up8_tiles[s][:, :, 1024 * i:1024 * (i + 1)],
                            up8_in[s, :, :, 1024 * i:1024 * (i + 1)])

                    ngate = [0]

                    def gate(src):
                        i = ngate[0]
                        ngate[0] += 1
                        nc.sync.dma_start(dly[:, 64 * i:64 * (i + 1)], src)

                    ch = upc_tiles
                    nc.sync.dma_start(ch[(s, 0)][:, :, 0:256],
                                      upc_in[s, 0, :, :, 0:256])
                    gate(xnb_t[:, 0, 0:64])
                    q8push(0)
                    nc.sync.dma_start(ch[(s, 0)][:, :, 256:512],
                                      upc_in[s, 0, :, :, 256:512])
                    gate(ch[(s, 0)][:, 0, 0:64])
                    nc.sync.dma_start(ch[(s, 1)][:], upc_in[s, 1])
                    gate(ch[(s, 0)][:, 0, 256:320])
                    q8push(1)
                    nc.sync.dma_start(ch[(s, 2)][:], upc_in[s, 2])
                    nc.sync.dma_start(ch[(s, 3)][:], upc_in[s, 3])
                    gate(ch[(s, 1)][:, 0, 0:64])
                    q8push(2)
                    nc.sync.dma_start(ch[(s, 4)][:], upc_in[s, 4])
                    nc.sync.dma_start(ch[(s, 5)][:], upc_in[s, 5])
                    gate(ch[(s, 3)][:, 0, 0:64])
                    q8push(3)
                    nc.sync.dma_start(ch[(s, 6)][:], upc_in[s, 6])
                    nc.sync.dma_start(ch[(s, 7)][:], upc_in[s, 7])
                else:
                    nc.sync.dma_start(w8[:], up8_in[s])
                    for c in range(NCH):
                        nc.sync.dma_start(upc_tiles[(s, c)][:], upc_in[s, c])

            # initial DMA pushes (program order = sync-engine issue order;
            # all transfers share one ~400GB/s queue, so order = priority):
            # seg0 first tokens -> seg0 up weights -> seg0 down -> the rest.
            # first pushes go out on idle engines in parallel so the sync
            # engine's ~0.65us-per-descriptor serialization doesn't delay
            # the startup-critical transfers
            c0 = min(512, comp_tok[0])
            dly = warm_pool.tile([P, 64 * 12], bf16, tag="dly")
            nc.sync.dma_start(xnb_t[:, :, 0:c0], xnb_in[:, :, 0:c0])
            nc.sync.dma_start(xn8_t[:, :, 0:c0], xn8_in[:, :, 0:c0])
            push_seg_up(0, fine=True)
            dnb0 = dnb_pool.tile([P, KHB, 1024], bf16, tag="dnb", name="dnb_0")
            nc.sync.dma_start(dnb0[:], dnb_in[0])
            dnb_tiles = {0: dnb0}
            d8 = dn8_pool.tile([P, 2 * N8D, 1024], fp8, tag="dn8", name="dn8_0")
            nc.sync.dma_start(d8[:], dn8_in[0])
            dn8_tiles = {0: d8}
            if c0 < T:
                nc.sync.dma_start(xnb_t[:, :, c0:T], xnb_in[:, :, c0:T])
                nc.sync.dma_start(xn8_t[:, :, c0:T], xn8_in[:, :, c0:T])

            col = 0
            for s in range(K):
                # down bf16 weights: bufs=1, pushed here (s>0) so the WAR
                # wait on the previous segment's last down matmul is met.
                if s > 0:
                    dnbt = dnb_pool.tile([P, KHB, 1024], bf16, tag="dnb",
                                         name=f"dnb_{s}")
                    nc.sync.dma_start(dnbt[:], dnb_in[s])
                    dnb_tiles[s] = dnbt
                dnbt = dnb_tiles[s]

                stok = comp_tok[s]
                rem = stok
                segoff = 0
                while rem > 0:
                    gn = min(512, rem)
                    rem -= gn
                    is_last_group = rem == 0
                    xslc = slice(col + segoff, col + segoff + gn)
                    segoff += gn
                    act_t = act_pool.tile([P, 16, gn], bf16, tag="act")
                    act8_t = act8_pool.tile([P, 2 * N8D, gn], fp8, tag="act8")
                    # ---- up projection: 16 rounds of (a, g) pairs ----
                    for j in range(16):
                        ch = upc_tiles[(s, j // 2)]
                        off = (j % 2) * 256
                        pa = ps.tile([P, gn], f32, tag="ps", name="pa")
                        pg = ps.tile([P, gn], f32, tag="ps", name="pg")
                        for ko in range(KOB):
                            first = ko == 0
                            last = N8U == 0 and ko == KOB - 1
                            nc.tensor.matmul(pa[:], ch[:, ko, off:off + P],
                                             xnb_t[:, ko, xslc],
                                             start=first, stop=last)
                            nc.tensor.matmul(pg[:],
                                             ch[:, ko, off + P:off + 256],
                                             xnb_t[:, ko, xslc],
                                             start=first, stop=last)
                        w8 = up8_tiles[s]
                        for pr in range(N8U):
                            first = KOB == 0 and pr == 0
                            last = pr == N8U - 1
                            kk = slice(2 * pr, 2 * pr + 2)
                            ca = j * 256
                            nc.tensor.matmul(pa[:], w8[:, kk, ca:ca + P],
                                             xn8_t[:, kk, xslc],
                                             start=first, stop=last,
                                             perf_mode=DR)
                            nc.tensor.matmul(pg[:],
                                             w8[:, kk, ca + P:ca + 256],
                                             xn8_t[:, kk, xslc],
                                             start=first, stop=last,
                                             perf_mode=DR)
                        # swiglu: act = pa * silu(pg); fp8 kh's go to act8
                        nc.scalar.activation(act_t[:, j, :], pg[:], Silu)
                        if j < KHB:
                            nc.vector.tensor_mul(act_t[:, j, :], pa[:],
                                                 act_t[:, j, :])
                        else:
                            nc.vector.tensor_mul(act8_t[:, j - KHB, :], pa[:],
                                                 act_t[:, j, :])
                        # prefetch next segment's up weights as their chunk
                        # buffers free up (after this group's last use)
                        if is_last_group and s + 1 < K and j == 15:
                            push_seg_up(s + 1)
                            d8n = dn8_pool.tile([P, 2 * N8D, 1024], fp8,
                                                tag="dn8", name=f"dn8_{s+1}")
                            nc.sync.dma_start(d8n[:], dn8_in[s + 1])
                            dn8_tiles[s + 1] = d8n
                    # ---- down projection: 4 rounds x 2 dout tiles ----
                    d8t = dn8_tiles[s]
                    for rr in range(4):
                        pd = [ps.tile([P, gn], f32, tag="ps", name=f"pd{q}")
                              for q in range(2)]
                        for kh in range(KHB):
                            first = kh == 0
                            last = N8D == 0 and kh == KHB - 1
                            for q in range(2):
                                cc = (2 * rr + q) * P
                                nc.tensor.matmul(pd[q][:],
                                                 dnbt[:, kh, cc:cc + P],
                                                 act_t[:, kh, :],
                                                 start=first, stop=last)
                        for pr in range(N8D):
                            first = KHB == 0 and pr == 0
                            last = pr == N8D - 1
                            kk = slice(2 * pr, 2 * pr + 2)
                            for q in range(2):
                                cc = (2 * rr + q) * P
                                nc.tensor.matmul(pd[q][:],
                                                 d8t[:, kk, cc:cc + P],
                                                 act8_t[:, kk, :],
                                                 start=first, stop=last,
                                                 perf_mode=DR)
                        yc = yc_pool.tile([P, 2, gn], f32, tag="yc")
                        if s == K - 1 and is_last_group and rr == 3:
                            # final round: split the copy across DVE + ACT
                            # and DMA per half so the drain tail is short
                            nc.vector.tensor_copy(yc[:, 0, :], pd[0][:])
                            nc.sync.dma_start(yt_out[:, 6, xslc], yc[:, 0, :])
                            nc.scalar.activation(
                                yc[:, 1, :], pd[1][:],
                                mybir.ActivationFunctionType.Copy)
                            nc.sync.dma_start(yt_out[:, 7, xslc], yc[:, 1, :])
                        else:
                            for q in range(2):
                                nc.vector.tensor_copy(yc[:, q, :], pd[q][:])
                            nc.sync.dma_start(
                                yt_out[:, 2 * rr:2 * rr + 2, xslc], yc[:])
                col += stok

    _patch_bass_json(nc)
    return nc


# ----------------------------------------------------------------------------
# Host-side weight packing
# ----------------------------------------------------------------------------
def _pack_up(up_e):
    """up_w[e] [DIM, 2H] f32 -> (upc [8, P, KOB, 512] bf16,
    up8 [P, 2*N8U, 4096] fp8)."""
    Wd = np.empty((DIM, 4096), dtype=np.float32)
    for j in range(16):
        a = up_e[:, j * P:(j + 1) * P]
        if j >= KHB:
            a = a * (1.0 / SD)
        Wd[:, j * 256:j * 256 + P] = a
        Wd[:, j * 256 + P:(j + 1) * 256] = up_e[:, HID + j * P:HID + (j + 1) * P]
    ub = Wd[:KOB * P].astype(BF16).reshape(KOB, P, NCHU, 512)
    upc = np.ascontiguousarray(ub.transpose(2, 1, 0, 3))
    u8 = _q8(Wd[KOB * P:] * SU).reshape(2 * N8U, P, 4096)
    up8 = np.ascontiguousarray(u8.transpose(1, 0, 2))
    return upc, up8


NCHU = 8


def _pack_dn(dn_e):
    """down_w[e] [HID, DIM] f32 -> (dnb [P, KHB, 1024] bf16,
    dn8 [P, 2*N8D, 1024] fp8)."""
    db = dn_e[:KHB * P].astype(BF16).reshape(KHB, P, DIM)
    dnb = np.ascontiguousarray(db.transpose(1, 0, 2))
    d8 = _q8(dn_e[KHB * P:] * SD).reshape(2 * N8D, P, DIM)
    dn8 = np.ascontiguousarray(d8.transpose(1, 0, 2))
    return dnb, dn8


# ----------------------------------------------------------------------------
# Entry point
# ----------------------------------------------------------------------------
def _run(inputs, trace=False, tmpdir=None):
    from concourse.bass_utils import run_bass_kernel_spmd

    x = np.asarray(inputs["x"])
    scale = np.asarray(inputs["scale"])
    centroids = np.asarray(inputs["centroids"])
    up_w = np.asarray(inputs["up_w"])
    down_w = np.asarray(inputs["down_w"])

    B, S, D = x.shape
    ntok = B * S
    xf32 = x.reshape(ntok, D).astype(np.float32)

    xn, ids = _route(x, scale, centroids)
    comp_tok, slot_expert, chunks, tok_by_e = _plan(ids)
    K = len(comp_tok)
    T = sum(comp_tok)

    up_packed = {}
    dn_packed = {}
    for e in range(N_EXPERTS):
        if any(ee == e for ee in slot_expert.values()):
            up_packed[e] = _pack_up(up_w[e].astype(np.float32))
            dn_packed[e] = _pack_dn(down_w[e].astype(np.float32))

    xnT = np.ascontiguousarray(xn.T)  # [DIM, ntok] f32
    cursor = [0] * N_EXPERTS
    core_cols_tok = [np.zeros(T, dtype=np.int64) for _ in range(N_CORES)]
    core_cols_valid = [np.zeros(T, dtype=bool) for _ in range(N_CORES)]
    in_maps = []
    # fill order must match _plan's chunk assignment (capacity desc, core asc)
    fill_order = {}
    for e in range(N_EXPERTS):
        slots = [(c, j) for (c, j), ee in slot_expert.items() if ee == e]
        slots.sort(key=lambda s: (-comp_tok[s[1]], s[0]))
        fill_order[e] = slots
    seg_start = np.concatenate([[0], np.cumsum(comp_tok)])
    for e in range(N_EXPERTS):
        for (c, j) in fill_order[e]:
            take = chunks[(c, j)]
            if take:
                sel = tok_by_e[e][cursor[e]:cursor[e] + take]
                cursor[e] += take
                a = int(seg_start[j])
                core_cols_tok[c][a:a + take] = sel
                core_cols_valid[c][a:a + take] = True

    for c in range(N_CORES):
        upc = np.zeros((K, NCHU, P, KOB, 512), dtype=BF16)
        up8 = np.zeros((K, P, 2 * N8U, 4096), dtype=FP8)
        dnb = np.zeros((K, P, KHB, 1024), dtype=BF16)
        dn8 = np.zeros((K, P, 2 * N8D, 1024), dtype=FP8)
        for j in range(K):
            e = slot_expert[(c, j)]
            if e is not None:
                upc[j], up8[j] = up_packed[e]
                dnb[j], dn8[j] = dn_packed[e]
        xcols = xnT[:, core_cols_tok[c]]  # [DIM, T] f32 (invalid cols garbage)
        xcols = xcols * core_cols_valid[c][None, :]
        xnb = np.ascontiguousarray(
            xcols[:KOB * P].astype(BF16).reshape(KOB, P, T).transpose(1, 0, 2))
        xn8 = np.ascontiguousarray(
            _q8(xcols[KOB * P:] * (1.0 / SU)).reshape(2 * N8U, P, T)
            .transpose(1, 0, 2))
        in_maps.append({"xnb": xnb, "xn8": xn8, "upc": upc, "up8": up8,
                        "dnb": dnb, "dn8": dn8})

    nc = _build_program(comp_tok)
    kwargs = {}
    if trace:
        kwargs = dict(trace=True, tmpdir=tmpdir)
    res = run_bass_kernel_spmd(nc, in_maps, core_ids=list(range(N_CORES)),
                               **kwargs)

    # ---- scatter + skip ----
    out = xf32.copy()
    for c in range(N_CORES):
        yt = np.ascontiguousarray(
            res.results[c]["yt"].reshape(P, 8, T).transpose(1, 0, 2)
        ).reshape(8 * P, T)  # [DIM, T]
        valid = core_cols_valid[c]
        toks = core_cols_tok[c][valid]
        out[toks] = xf32[toks] + yt[:, valid].T
    return out.reshape(B, S, D).astype(x.dtype), res


def kernel(**inputs) -> np.ndarray:
    out, _ = _run(inputs)
    return out
